# revision 1
# baseline (speedup 1.0000x reference)
"""DeepseekV4-style sparse attention on 8 Trainium2 cores (Bass/Tile).

Sharding: data-parallel over batch (2) x tensor-parallel over heads (16 -> 4
groups of 4).  Core c handles batch c//4 and heads [4*(c%4), 4*(c%4)+4).
wkv/wgate/ape (single shared KV head) are replicated; each core computes the
pooled KV itself.  Per-core partial outputs (attn_heads @ wo_rows) are summed
on the host.

Device layout notes:
  - host passes hidden TRANSPOSED ([HID, S]) and cast to bf16 so every matmul
    has its contraction dim on partitions with no on-device transposes
  - q is produced directly in qT layout [head_dim, t] (matmul lhsT = wq)
  - scores are computed transposed (S^T[w, q]); softmax sums over w via a
    ones-vector matmul, PV consumes exp(S^T) directly, and the resulting
    attnT [hd, t] is exactly the lhsT the output projection wants
  - causal structure: query chunk j (512 queries) sees w-chunks 0..j; only
    the diagonal chunk needs a mask, identical for every j (precomputed 0/1)
  - all RoPE math runs on partitions 64..127 so no op shifts partition bases
"""

import numpy as np
import ml_dtypes

import concourse.bass as bass
import concourse.mybir as mybir
import concourse.tile as tile
from concourse.bass import ts
from concourse.masks import make_identity

F32 = mybir.dt.float32
BF16 = mybir.dt.bfloat16
AF = mybir.ActivationFunctionType

# Problem constants (hardcoded per the harness contract).
B, S, HID, NH, HD, RD, RATIO = 2, 4096, 2048, 16, 128, 64, 4
THETA = 10000.0
NW = S // RATIO              # 1024 pooled windows
N_CORES = 8
HPC = 4                      # heads per core
CW = HPC * HD                # per-core q/wo width (512)
TCH = 512                    # t-chunk size
NCH = S // TCH               # 8 t-chunks
WCH = 128                    # w-chunk size
KCH = HID // 128             # 16 contraction chunks
SCALE = HD ** -0.5

_PAIR_SWAP = [i ^ 1 for i in range(32)]


def _build_nc(n_reps: int = 1, split_waits: bool = True):
    nc = bass.Bass()
    dp = nc.declare_dram_parameter
    ht = dp("ht", [HID, S], BF16, isOutput=False)
    wq = dp("wq", [HID, CW], BF16, isOutput=False)
    wkv = dp("wkv", [HID, 2 * HD], BF16, isOutput=False)
    wg = dp("wg", [HID, 2 * HD], BF16, isOutput=False)
    wo = dp("wo", [CW, HID], BF16, isOutput=False)
    eape = dp("eape", [HD, 2 * RATIO], F32, isOutput=False)
    esink = dp("esink", [1, HPC], F32, isOutput=False)
    cosq = dp("cosq", [RD, S], BF16, isOutput=False)
    sinq = dp("sinq", [RD, S], BF16, isOutput=False)
    cosk = dp("cosk", [RD, NW], BF16, isOutput=False)
    sink = dp("sink", [RD, NW], BF16, isOutput=False)
    bandm = dp("bandm", [WCH, TCH], BF16, isOutput=False)
    out = dp("out", [S, HID], F32, isOutput=True)
    args = (ht, wq, wkv, wg, wo, eape, esink, cosq, sinq, cosk, sink, bandm, out)

    with tile.TileContext(nc) as tc:
        if n_reps > 1:
            with tc.For_i(0, n_reps, 1):
                _body(nc, tc, *args)
        else:
            _body(nc, tc, *args)
    if split_waits:
        _split_multi_waits(nc)
    return nc


def _body(nc, tc, ht, wq, wkv, wg, wo, eape, esink,
          cosq, sinq, cosk, sink, bandm, out):
    with tc.tile_pool(name="persist", bufs=1) as pp:
        # ---- persistent SBUF state ----
        qT = [pp.tile([128, S], BF16, tag=f"qT{m}", name=f"qT{m}") for m in range(HPC)]
        kvlo = pp.tile([HD, RATIO + S], BF16, tag="kvlo", name="kvlo")
        kvhi = pp.tile([HD, S], BF16, tag="kvhi", name="kvhi")
        glo = pp.tile([HD, RATIO + S], BF16, tag="glo", name="glo")
        ghi = pp.tile([HD, S], BF16, tag="ghi", name="ghi")
        # rope tables live on partitions 64..127 (matching the rope rows)
        cosq_s = pp.tile([128, S], BF16, tag="cosq", name="cosq")
        sinq_s = pp.tile([128, S], BF16, tag="sinq", name="sinq")
        cosk_s = pp.tile([128, NW], BF16, tag="cosk", name="cosk")
        sink_s = pp.tile([128, NW], BF16, tag="sink", name="sink")
        eape_s = pp.tile([HD, 2 * RATIO], F32, tag="eape", name="eape")
        esink_s = pp.tile([1, HPC], F32, tag="esink", name="esink")
        bandm_s = pp.tile([WCH, TCH], BF16, tag="bandm", name="bandm")
        wo_s = pp.tile([HD, HPC, HID], BF16, tag="wo", name="wo")
        ones_w = pp.tile([WCH, 1], BF16, tag="ones_w", name="ones_w")
        ones_p = pp.tile([1, HD], F32, tag="ones_p", name="ones_p")
        kT = pp.tile([HD, NW], BF16, tag="kT", name="kT")
        v_s = pp.tile([WCH, NW // WCH, HD], BF16, tag="v", name="v")
        pooledT = pp.tile([HD, NW], F32, tag="pooledT", name="pooledT")
        ident = pp.tile([128, 128], F32, tag="ident", name="ident")

        nc.sync.dma_start(cosq_s[RD:128, :], cosq[:])
        nc.sync.dma_start(sinq_s[RD:128, :], sinq[:])
        nc.sync.dma_start(cosk_s[RD:128, :], cosk[:])
        nc.sync.dma_start(sink_s[RD:128, :], sink[:])
        nc.sync.dma_start(eape_s[:], eape[:])
        nc.sync.dma_start(esink_s[:], esink[:])
        nc.sync.dma_start(bandm_s[:], bandm[:])
        nc.sync.dma_start(wo_s[:], wo.rearrange("(h p) e -> p h e", p=HD))
        nc.vector.memset(ones_w[:], 1.0)
        nc.vector.memset(ones_p[:], 1.0)
        nc.gpsimd.memset(kvlo[:, 0:RATIO], 0.0)
        nc.gpsimd.memset(glo[:, 0:RATIO], -30000.0)
        make_identity(nc, ident[:])

        # ---- stage A: projections (q with fused RoPE, kv, gate) ----
        with (
            tc.tile_pool(name="wts", bufs=1) as wts,
            tc.tile_pool(name="hslab", bufs=2) as hs,
            tc.tile_pool(name="evict", bufs=3) as ev,
            tc.tile_pool(name="psA", bufs=6, space="PSUM") as psA,
        ):
            wq_s = wts.tile([128, KCH, CW], BF16, tag="wq", name="wq")
            wkv_s = wts.tile([128, KCH, 2 * HD], BF16, tag="wkv", name="wkv")
            wg_s = wts.tile([128, KCH, 2 * HD], BF16, tag="wg", name="wg")
            nc.sync.dma_start(wq_s[:], wq.rearrange("(k p) c -> p k c", p=128))
            nc.sync.dma_start(wkv_s[:], wkv.rearrange("(k p) c -> p k c", p=128))
            nc.sync.dma_start(wg_s[:], wg.rearrange("(k p) c -> p k c", p=128))

            for j in range(NCH):
                tsl = ts(j, TCH)
                hsl = hs.tile([128, KCH, TCH], BF16, tag="hslab", name="hslab")
                nc.sync.dma_start(
                    hsl[:], ht[:, tsl].rearrange("(k p) t -> p k t", p=128))

                # q projection, one head (=one 128-col chunk of wq) at a time
                for m in range(HPC):
                    ps = psA.tile([128, TCH], F32, tag="psA", name="psA")
                    for k in range(KCH):
                        nc.tensor.matmul(ps[:], wq_s[:, k, ts(m, 128)],
                                         hsl[:, k, :], start=(k == 0),
                                         stop=(k == KCH - 1))
                    # rows 0:64 pass through; rows 64:128 interleaved RoPE
                    nc.scalar.copy(qT[m][0:RD, tsl], ps[0:RD, :])
                    rb = ev.tile([128, TCH], BF16, tag="ropebuf", name="ropebuf")
                    nc.scalar.copy(rb[RD:128, :], ps[RD:128, :])
                    sw = ev.tile([128, TCH], BF16, tag="ropeswap", name="ropeswap")
                    nc.vector.stream_shuffle(sw[RD:128, :], rb[RD:128, :],
                                             _PAIR_SWAP)
                    t1 = ev.tile([128, TCH], BF16, tag="ropet1", name="ropet1")
                    nc.vector.tensor_mul(t1[RD:128, :], rb[RD:128, :],
                                         cosq_s[RD:128, tsl])
                    t2 = ev.tile([128, TCH], BF16, tag="ropet2", name="ropet2")
                    nc.vector.tensor_mul(t2[RD:128, :], sw[RD:128, :],
                                         sinq_s[RD:128, tsl])
                    nc.vector.tensor_add(qT[m][RD:128, tsl], t1[RD:128, :],
                                         t2[RD:128, :])

                # kv / gate projections (lo = features 0:128, hi = 128:256)
                for dst, lo, wsrc in (
                    (kvlo, True, wkv_s), (kvhi, False, wkv_s),
                    (glo, True, wg_s), (ghi, False, wg_s),
                ):
                    ps = psA.tile([128, TCH], F32, tag="psA", name="psA")
                    col = slice(0, 128) if lo else slice(128, 256)
                    for k in range(KCH):
                        nc.tensor.matmul(ps[:], wsrc[:, k, col], hsl[:, k, :],
                                         start=(k == 0), stop=(k == KCH - 1))
                    if lo:
                        nc.scalar.copy(
                            dst[:, RATIO + j * TCH:RATIO + (j + 1) * TCH], ps[:])
                    else:
                        nc.scalar.copy(dst[:, tsl], ps[:])

        # ---- stage B: overlap gated pooling -> pooledT / kT / V ----
        with (
            tc.tile_pool(name="poolb", bufs=1) as pb,
            tc.tile_pool(name="psB", bufs=4, space="PSUM") as psB,
        ):
            numer = pb.tile([HD, NW], F32, tag="numer", name="numer")
            denom = pb.tile([HD, NW], F32, tag="denom", name="denom")
            for half, (g_src, kv_src, acol) in enumerate(
                ((glo, kvlo, slice(0, RATIO)),
                 (ghi, kvhi, slice(RATIO, 2 * RATIO)))
            ):
                n = (RATIO + S) if half == 0 else S
                e = pb.tile([HD, RATIO + S], F32, tag="poole", name="poole")
                nc.scalar.activation(e[:, 0:n], g_src[:, 0:n], AF.Exp)
                nc.vector.tensor_mul(
                    e[:, 0:n].rearrange("d (w r) -> d w r", r=RATIO),
                    e[:, 0:n].rearrange("d (w r) -> d w r", r=RATIO),
                    eape_s[:, None, acol].to_broadcast([HD, n // RATIO, RATIO]))
                ea = pb.tile([HD, RATIO + S], F32, tag="poolea", name="poolea")
                nc.vector.tensor_mul(ea[:, 0:n], e[:, 0:n], kv_src[:, 0:n])
                # windowed sums over the 4 slots via strided adds
                for acc, src in ((denom, e), (numer, ea)):
                    s3 = src[:, 0:S].rearrange("d (w r) -> d w r", r=RATIO)
                    nm = f"poolred{half}{1 if acc is numer else 0}"
                    ra = pb.tile([HD, NW], F32, tag="poolra", name=nm + "a")
                    nc.vector.tensor_add(ra[:], s3[:, :, 0], s3[:, :, 1])
                    rc = pb.tile([HD, NW], F32, tag="poolrc", name=nm + "c")
                    nc.vector.tensor_add(rc[:], s3[:, :, 2], s3[:, :, 3])
                    if half == 0:
                        nc.vector.tensor_add(acc[:], ra[:], rc[:])
                    else:
                        nc.vector.tensor_add(ra[:], ra[:], rc[:])
                        nc.vector.tensor_add(acc[:], acc[:], ra[:])
            rec = pb.tile([HD, NW], F32, tag="poolrec", name="poolrec")
            nc.vector.reciprocal(rec[:], denom[:])
            nc.vector.tensor_mul(pooledT[:], numer[:], rec[:])

            # V = pooledT^T (PE transpose, 128x128 blocks), bf16
            for wb in range(NW // WCH):
                tp = psB.tile([128, 128], F32, tag="vtrans", name="vtrans")
                nc.tensor.transpose(tp[:], pooledT[:, ts(wb, 128)], ident[:])
                nc.scalar.copy(v_s[:, wb, :], tp[:])

            # kT = partial interleaved RoPE of pooledT at positions w*RATIO
            nc.scalar.copy(kT[0:RD, :], pooledT[0:RD, :])
            krb = pb.tile([128, NW], BF16, tag="krope", name="krope")
            nc.scalar.copy(krb[RD:128, :], pooledT[RD:128, :])
            ksw = pb.tile([128, NW], BF16, tag="kswap", name="kswap")
            nc.vector.stream_shuffle(ksw[RD:128, :], krb[RD:128, :], _PAIR_SWAP)
            kt1 = pb.tile([128, NW], BF16, tag="kt1", name="kt1")
            nc.vector.tensor_mul(kt1[RD:128, :], krb[RD:128, :],
                                 cosk_s[RD:128, :])
            kt2 = pb.tile([128, NW], BF16, tag="kt2", name="kt2")
            nc.vector.tensor_mul(kt2[RD:128, :], ksw[RD:128, :],
                                 sink_s[RD:128, :])
            nc.vector.tensor_add(kT[RD:128, :], kt1[RD:128, :], kt2[RD:128, :])

        # ---- stage C: attention + output projection, per q-chunk ----
        with (
            tc.tile_pool(name="pt", bufs=4) as ptp,
            tc.tile_pool(name="att", bufs=2) as att,
            tc.tile_pool(name="psS", bufs=3, space="PSUM") as psS,
            tc.tile_pool(name="psRB", bufs=1, space="PSUM") as psrb,
            tc.tile_pool(name="psO", bufs=2, space="PSUM") as pso,
            tc.tile_pool(name="psAcc", bufs=1, space="PSUM") as psacc,
        ):
            for j in range(NCH):
                tsl = ts(j, TCH)
                attnT = []
                for h in range(HPC):
                    qsl = qT[h][:, tsl]
                    den_ps = psacc.tile([1, TCH], F32, tag="den", name="den")
                    acc_ps = psacc.tile([128, TCH], F32, tag="acc", name="acc")
                    # score matmuls run one w-chunk ahead of the exp/sum/PV
                    # consumers so PE never sits behind the ACT exp
                    s_tiles = {}

                    def _score(wc, s_tiles=s_tiles, qsl=qsl):
                        sp = psS.tile([WCH, TCH], F32, tag="s", name="s")
                        nc.tensor.matmul(sp[:], kT[:, ts(wc, WCH)], qsl,
                                         start=True, stop=True)
                        s_tiles[wc] = sp

                    _score(0)
                    for wc in range(j + 1):
                        if wc < j:
                            _score(wc + 1)
                        s_ps = s_tiles.pop(wc)
                        pt = ptp.tile([WCH, TCH], BF16, tag="pt", name="pt")
                        nc.scalar.activation(pt[:], s_ps[:], AF.Exp,
                                             scale=SCALE)
                        if wc == j:
                            ptm = ptp.tile([WCH, TCH], BF16, tag="ptm", name="ptm")
                            nc.vector.tensor_mul(ptm[:], pt[:], bandm_s[:])
                            pt = ptm
                        nc.tensor.matmul(den_ps[:], ones_w[:], pt[:],
                                         start=(wc == 0), stop=(wc == j))
                        nc.tensor.matmul(acc_ps[:], v_s[:, wc, :], pt[:],
                                         start=(wc == 0), stop=(wc == j))
                    den_sb = att.tile([1, TCH], F32, tag="den_sb", name="den_sb")
                    nc.scalar.activation(den_sb[:], den_ps[:], AF.Identity,
                                         bias=esink_s[0:1, h:h + 1])
                    rec_sb = att.tile([1, TCH], F32, tag="rec_sb", name="rec_sb")
                    nc.vector.reciprocal(rec_sb[:], den_sb[:])
                    rb_ps = psrb.tile([128, TCH], F32, tag="rb", name="rb")
                    nc.tensor.matmul(rb_ps[:], ones_p[:], rec_sb[:],
                                     start=True, stop=True)
                    rb_sb = att.tile([128, TCH], F32, tag="rb_sb", name="rb_sb")
                    nc.scalar.copy(rb_sb[:], rb_ps[:])
                    a_sb = att.tile([128, TCH], BF16, tag=f"attnT{h}", name=f"attnT{h}")
                    nc.vector.tensor_mul(a_sb[:], acc_ps[:], rb_sb[:])
                    attnT.append(a_sb)

                for tt in range(TCH // 128):
                    for e in range(HID // TCH):
                        o_ps = pso.tile([128, TCH], F32, tag="o", name="o")
                        for h in range(HPC):
                            nc.tensor.matmul(o_ps[:], attnT[h][:, ts(tt, 128)],
                                             wo_s[:, h, ts(e, TCH)],
                                             start=(h == 0), stop=(h == HPC - 1))
                        o_sb = att.tile([128, TCH], F32, tag="o_sb", name="o_sb")
                        if e % 2 == 0:
                            nc.scalar.copy(o_sb[:], o_ps[:])
                        else:
                            nc.vector.tensor_copy(o_sb[:], o_ps[:])
                        nc.sync.dma_start(
                            out[j * TCH + tt * 128:j * TCH + (tt + 1) * 128,
                                ts(e, TCH)], o_sb[:])


_WS_CTR = [0]


def _split_multi_waits(nc):
    """This walrus build accepts at most ONE sync wait per instruction; hoist
    extras onto same-engine NOPs placed immediately before."""
    f = nc.m.functions[0]
    for blk in f.blocks:
        insts = blk.instructions
        if not any(i.sync_info is not None and len(i.sync_info.on_wait) > 1
                   for i in insts):
            continue
        new_list = []
        for inst in insts:
            si = inst.sync_info
            if si is not None and len(si.on_wait) > 1:
                waits = list(si.on_wait)
                for w in waits[:-1]:
                    _WS_CTR[0] += 1
                    new_list.append(mybir.InstNoOp(
                        name=f"waitsplit-{_WS_CTR[0]}",
                        engine=inst.engine,
                        bass_nofuse=True,
                        sync_info=mybir.SyncInfo(on_wait=[w], on_update=[])))
                inst.sync_info = mybir.SyncInfo(
                    on_wait=[waits[-1]], on_update=list(si.on_update))
            new_list.append(inst)
        blk.instructions = new_list


# ---------------------------------------------------------------------------
# host side
# ---------------------------------------------------------------------------

def _rope_tables(positions):
    half = RD // 2
    inv_freq = 1.0 / (THETA ** (np.arange(half, dtype=np.float64) / half))
    ang = positions[None, :].astype(np.float64) * inv_freq[:, None]  # [32, L]
    cos_t = np.repeat(np.cos(ang), 2, axis=0).astype(np.float32)
    sin_t = np.repeat(np.sin(ang), 2, axis=0).astype(np.float32)
    sin_t[0::2] *= -1.0                                  # a-rows get -sin
    return cos_t, sin_t


def _prep_inputs(hidden, wq, wkv, wgate, ape, sinks, wo):
    bf = ml_dtypes.bfloat16
    cosq_t, sinq_t = _rope_tables(np.arange(S))
    cosk_t, sink_t = _rope_tables(np.arange(NW) * RATIO)
    pw, ft = np.meshgrid(np.arange(WCH), np.arange(TCH), indexing="ij")
    band = (ft >= RATIO * pw + RATIO - 1).astype(np.float32)     # [WCH, TCH]
    eape = np.empty((HD, 2 * RATIO), np.float32)
    for r in range(RATIO):
        eape[:, r] = np.exp(ape[r, :HD])
        eape[:, RATIO + r] = np.exp(ape[r, HD:])
    maps = []
    for c in range(N_CORES):
        b, g = divmod(c, HPC)
        maps.append({
            "ht": np.ascontiguousarray(hidden[b].T).astype(bf),
            "wq": np.ascontiguousarray(wq[:, g * CW:(g + 1) * CW]).astype(bf),
            "wkv": wkv.astype(bf),
            "wg": wgate.astype(bf),
            "wo": np.ascontiguousarray(wo[g * CW:(g + 1) * CW, :]).astype(bf),
            "eape": eape,
            "esink": np.exp(sinks[g * HPC:(g + 1) * HPC]).astype(
                np.float32).reshape(1, HPC),
            "cosq": cosq_t.astype(bf), "sinq": sinq_t.astype(bf),
            "cosk": cosk_t.astype(bf), "sink": sink_t.astype(bf),
            "bandm": band.astype(bf),
        })
    return maps


_RUNNER_CACHE = {}


def _get_runner(n_reps: int = 1):
    if n_reps in _RUNNER_CACHE:
        return _RUNNER_CACHE[n_reps]
    import jax
    from jax.sharding import Mesh, PartitionSpec
    from jax.experimental.shard_map import shard_map
    from concourse.bass2jax import (_bass_exec_p, install_neuronx_cc_hook,
                                    partition_id_tensor)

    nc = _build_nc(n_reps)
    install_neuronx_cc_hook()
    partition_name = nc.partition_id_tensor.name if nc.partition_id_tensor else None
    in_names, out_names, out_avals, zero_outs = [], [], [], []
    for alloc in nc.m.functions[0].allocations:
        if not isinstance(alloc, mybir.MemoryLocationSet):
            continue
        name = alloc.memorylocations[0].name
        if alloc.kind == "ExternalInput":
            if name != partition_name:
                in_names.append(name)
        elif alloc.kind == "ExternalOutput":
            out_names.append(name)
            shape = tuple(alloc.tensor_shape)
            dtype = mybir.dt.np(alloc.dtype)
            out_avals.append(jax.core.ShapedArray(shape, dtype))
            zero_outs.append(np.zeros(shape, dtype))
    n_params = len(in_names)
    all_in_names = list(in_names) + out_names
    if partition_name is not None:
        all_in_names.append(partition_name)

    def _kernel_body(*args):
        operands = list(args)
        if partition_name is not None:
            operands.append(partition_id_tensor())
        outs = _bass_exec_p.bind(
            *operands,
            out_avals=tuple(out_avals),
            in_names=tuple(all_in_names),
            out_names=tuple(out_names),
            lowering_input_output_aliases=(),
            sim_require_finite=True,
            sim_require_nnan=True,
            nc=nc,
        )
        return tuple(outs)

    devices = jax.devices()[:N_CORES]
    mesh = Mesh(np.asarray(devices), ("core",))
    spec = PartitionSpec("core")
    fn = jax.jit(shard_map(
        _kernel_body, mesh=mesh,
        in_specs=(spec,) * (n_params + len(out_names)),
        out_specs=(spec,) * len(out_names), check_rep=False))
    runner = (fn, in_names, out_names, zero_outs, mesh)
    _RUNNER_CACHE[n_reps] = runner
    return runner


def _run_core_maps(maps, n_reps: int = 1):
    import jax
    from jax.sharding import NamedSharding, PartitionSpec
    fn, in_names, out_names, zero_outs, mesh = _get_runner(n_reps)
    sh = NamedSharding(mesh, PartitionSpec("core"))
    args = [jax.device_put(
        np.concatenate([np.asarray(m[name]) for m in maps], axis=0), sh)
        for name in in_names]
    for z in zero_outs:
        args.append(jax.device_put(
            np.zeros((N_CORES * z.shape[0], *z.shape[1:]), z.dtype), sh))
    res = fn(*args)
    jax.block_until_ready(res)
    return np.asarray(res[0]).reshape(N_CORES, S, HID)


def kernel(hidden, wq, wkv, wgate, ape, sinks, wo,
           ratio=RATIO, head_dim=HD, rope_head_dim=RD, num_heads=NH):
    hidden = np.asarray(hidden, np.float32)
    maps = _prep_inputs(hidden, np.asarray(wq, np.float32),
                        np.asarray(wkv, np.float32),
                        np.asarray(wgate, np.float32),
                        np.asarray(ape, np.float32),
                        np.asarray(sinks, np.float32),
                        np.asarray(wo, np.float32))
    partials = _run_core_maps(maps)
    out = np.empty((B, S, HID), np.float32)
    for b in range(B):
        out[b] = partials[b * HPC:(b + 1) * HPC].astype(np.float64).sum(
            axis=0).astype(np.float32)
    return out



# revision 3
# speedup vs baseline: 1.3503x; 1.3503x over previous
"""DeepseekV4-style sparse attention on 8 Trainium2 cores (Bass/Tile), v2.

Sharding: data-parallel over batch (2) x tensor-parallel over heads (16 -> 4
groups of 4).  Core c handles batch c//4 and heads [4*(c%4), 4*(c%4)+4).
NEW in v2: the kv/gate projections + gated pooling are sharded over the 4
cores of each batch group (each core computes its own S/4 token slice plus a
4-token halo, pools its 256 windows, and the groups AllGather the pooled
windows in bf16).  Per-core partial outputs (attn_heads @ wo_rows) are
summed on the host.

Issue-order pipeline (PE never idles for long):
  prologue: kv/gate shard projections -> pooling -> AllGather (on TOPSP/SDMA,
            overlapped) -> q projections for chunks 0,1 -> kT rope + V
            transposes
  main loop over q-chunks j, heads h:
            score/exp/den/PV stream for (j,h), then the *deferred* epilogue
            of the previous stream, then one q-projection unit for chunk j+2
            and one out-projection quarter for chunk j-1 as PE filler.
PSUM budget (8 banks): qproj 1 + scores 2 + acc 2 + den 1 + out/rb 2.
"""

import numpy as np
import ml_dtypes

import concourse.bass as bass
import concourse.mybir as mybir
import concourse.tile as tile
from concourse.bass import ts
from concourse.masks import make_identity

F32 = mybir.dt.float32
BF16 = mybir.dt.bfloat16
AF = mybir.ActivationFunctionType

# Problem constants (hardcoded per the harness contract).
B, S, HID, NH, HD, RD, RATIO = 2, 4096, 2048, 16, 128, 64, 4
THETA = 10000.0
NW = S // RATIO              # 1024 pooled windows
N_CORES = 8
GSZ = 4                      # cores per batch group (kv/gate shard factor)
TL = S // GSZ                # own token slice (1024)
WL = NW // GSZ               # own window slice (256)
HKV = TL + RATIO             # htkv cols: 4-token halo + own slice
HPC = 4                      # heads per core
CW = HPC * HD                # per-core q/wo width (512)
TCH = 512                    # t-chunk size
NCH = S // TCH               # 8 t-chunks
WCH = 128                    # w-chunk size
KCH = HID // 128             # 16 contraction chunks
SCALE = HD ** -0.5

_PAIR_SWAP = [i ^ 1 for i in range(32)]


def _build_nc(n_reps: int = 1, split_waits: bool = True):
    nc = bass.Bass(num_devices=N_CORES)
    dp = nc.declare_dram_parameter
    ht = dp("ht", [HID, S], BF16, isOutput=False)
    htkv = dp("htkv", [HID, HKV], BF16, isOutput=False)
    wq = dp("wq", [HID, CW], BF16, isOutput=False)
    wkv = dp("wkv", [HID, 2 * HD], BF16, isOutput=False)
    wg = dp("wg", [HID, 2 * HD], BF16, isOutput=False)
    wo = dp("wo", [CW, HID], BF16, isOutput=False)
    eape = dp("eape", [HD, 2 * RATIO], F32, isOutput=False)
    esink = dp("esink", [1, HPC], F32, isOutput=False)
    gfix = dp("gfix", [HD, 1], F32, isOutput=False)
    cosq = dp("cosq", [RD, S], BF16, isOutput=False)
    sinq = dp("sinq", [RD, S], BF16, isOutput=False)
    cosk = dp("cosk", [RD, NW], BF16, isOutput=False)
    sink = dp("sink", [RD, NW], BF16, isOutput=False)
    bandm = dp("bandm", [WCH, TCH], BF16, isOutput=False)
    out = dp("out", [S, HID], BF16, isOutput=True)
    args = (ht, htkv, wq, wkv, wg, wo, eape, esink, gfix,
            cosq, sinq, cosk, sink, bandm, out)

    # collectives cannot sit inside a hardware For_i loop, so timing builds
    # replicate the whole body as straight-line code instead
    with tile.TileContext(nc) as tc:
        for r in range(n_reps):
            _body(nc, tc, *args, rep=r)
    if split_waits:
        _split_multi_waits(nc)
    return nc


def _body(nc, tc, ht, htkv, wq, wkv, wg, wo, eape, esink, gfix,
          cosq, sinq, cosk, sink, bandm, out, rep=0):
    cc_in = nc.dram_tensor(f"cc_in{rep}", [128, WL], F32)
    cc_out = nc.dram_tensor(f"cc_out{rep}", [GSZ, 128, WL], F32)

    with (
        tc.tile_pool(name="persist", bufs=1) as pp,
        tc.tile_pool(name="hslab", bufs=2) as hs,
    ):
        # ---- persistent SBUF state ----
        qT = [pp.tile([128, S], BF16, tag=f"qT{m}", name=f"qT{m}")
              for m in range(HPC)]
        kvlo = pp.tile([HD, HKV], BF16, tag="kvlo", name="kvlo")
        kvhi = pp.tile([HD, TL], BF16, tag="kvhi", name="kvhi")
        glo = pp.tile([HD, HKV], BF16, tag="glo", name="glo")
        ghi = pp.tile([HD, TL], BF16, tag="ghi", name="ghi")
        cosq_s = pp.tile([128, S], BF16, tag="cosq", name="cosq")
        sinq_s = pp.tile([128, S], BF16, tag="sinq", name="sinq")
        cosk_s = pp.tile([128, NW], BF16, tag="cosk", name="cosk")
        sink_s = pp.tile([128, NW], BF16, tag="sink", name="sink")
        eape_s = pp.tile([HD, 2 * RATIO], F32, tag="eape", name="eape")
        esink_s = pp.tile([1, HPC], F32, tag="esink", name="esink")
        gfix_s = pp.tile([HD, 1], F32, tag="gfix", name="gfix")
        bandm_s = pp.tile([WCH, TCH], BF16, tag="bandm", name="bandm")
        wo_s = pp.tile([HD, HPC, HID], BF16, tag="wo", name="wo")
        wq_s = pp.tile([128, KCH, CW], BF16, tag="wq", name="wq")
        wkv_s = pp.tile([128, KCH, 2 * HD], BF16, tag="wkv", name="wkv")
        wg_s = pp.tile([128, KCH, 2 * HD], BF16, tag="wg", name="wg")
        ones_w = pp.tile([WCH, 1], BF16, tag="ones_w", name="ones_w")
        ones_p = pp.tile([1, HD], F32, tag="ones_p", name="ones_p")
        kT = pp.tile([HD, NW], BF16, tag="kT", name="kT")
        v_s = pp.tile([WCH, NW // WCH, HD], BF16, tag="v", name="v")
        pg = pp.tile([128, NW], F32, tag="pg", name="pg")
        ident = pp.tile([128, 128], F32, tag="ident", name="ident")

        # DMA issue order is queue order: kv/gate weights + small tables
        # first so the PE can start within ~6us; bulky tables later.
        nc.sync.dma_start(wkv_s[:], wkv.rearrange("(k p) c -> p k c", p=128))
        nc.sync.dma_start(wg_s[:], wg.rearrange("(k p) c -> p k c", p=128))
        nc.sync.dma_start(eape_s[:], eape[:])
        nc.sync.dma_start(esink_s[:], esink[:])
        nc.sync.dma_start(gfix_s[:], gfix[:])
        nc.sync.dma_start(bandm_s[:], bandm[:])
        nc.vector.memset(ones_w[:], 1.0)
        nc.vector.memset(ones_p[:], 1.0)
        make_identity(nc, ident[:])

        def load_hsl(j):
            hsl = hs.tile([128, KCH, TCH], BF16, tag="hslab", name="hslab",
                          bufs=2)
            nc.sync.dma_start(
                hsl[:], ht[:, ts(j, TCH)].rearrange("(k p) t -> p k t", p=128))
            return hsl

        def qp_unit(psA, hsl, j, m, qp_bufs=1):
            """q projection for head m, chunk j, with fused interleaved RoPE."""
            tsl = ts(j, TCH)
            ps = psA.tile([128, TCH], F32, tag="psA", name="psA",
                          bufs=qp_bufs)
            for k in range(KCH):
                nc.tensor.matmul(ps[:], wq_s[:, k, ts(m, 128)], hsl[:, k, :],
                                 start=(k == 0), stop=(k == KCH - 1))
            nc.scalar.copy(qT[m][0:RD, tsl], ps[0:RD, :])
            rb = hs.tile([128, TCH], BF16, tag="ropebuf", name="ropebuf")
            nc.scalar.copy(rb[RD:128, :], ps[RD:128, :])
            sw = hs.tile([128, TCH], BF16, tag="ropeswap", name="ropeswap")
            nc.vector.stream_shuffle(sw[RD:128, :], rb[RD:128, :], _PAIR_SWAP)
            t1 = hs.tile([128, TCH], BF16, tag="ropet1", name="ropet1")
            nc.vector.tensor_mul(t1[RD:128, :], rb[RD:128, :],
                                 cosq_s[RD:128, tsl])
            t2 = hs.tile([128, TCH], BF16, tag="ropet2", name="ropet2")
            nc.vector.tensor_mul(t2[RD:128, :], sw[RD:128, :],
                                 sinq_s[RD:128, tsl])
            nc.vector.tensor_add(qT[m][RD:128, tsl], t1[RD:128, :],
                                 t2[RD:128, :])

        # =============== prologue scope ===============
        with (
            tc.tile_pool(name="psP", bufs=3, space="PSUM") as psP,
            tc.tile_pool(name="poolb", bufs=1) as pb,
        ):
            # ---- kv/gate shard projections (own slice + 4-token halo) ----
            hh = pb.tile([128, KCH, RATIO], BF16, tag="hhalo", name="hhalo")
            nc.sync.dma_start(
                hh[:], htkv[:, 0:RATIO].rearrange("(k p) t -> p k t", p=128))
            hks = []
            for e in range(TL // TCH):
                hk = hs.tile([128, KCH, TCH], BF16, tag="hslab", name="hslab",
                             bufs=2)
                nc.sync.dma_start(
                    hk[:], htkv[:, RATIO + e * TCH: RATIO + (e + 1) * TCH]
                    .rearrange("(k p) t -> p k t", p=128))
                hks.append(hk)
            hsl_cache = {}
            for j in range(2):
                hsl_cache[j] = load_hsl(j)
            nc.sync.dma_start(wq_s[:], wq.rearrange("(k p) c -> p k c", p=128))
            for j in range(2, 4):
                hsl_cache[j] = load_hsl(j)

            def kvg_group(dst, col, wsrc, dslice, rhs):
                ps = psP.tile([128, TCH], F32, tag="psP", name="psP")
                n = rhs.shape[-1]
                for k in range(KCH):
                    nc.tensor.matmul(ps[:, 0:n], wsrc[:, k, col], rhs[:, k, :],
                                     start=(k == 0), stop=(k == KCH - 1))
                if dst is glo and dslice.start == 0:
                    nc.scalar.activation(dst[:, dslice], ps[:, 0:n],
                                         AF.Identity, bias=gfix_s[:, 0:1])
                else:
                    nc.scalar.copy(dst[:, dslice], ps[:, 0:n])

            # halo kv first (only needs wkv + the tiny halo slab)
            kvg_group(kvlo, slice(0, 128), wkv_s, slice(0, RATIO), hh)
            for e in range(TL // TCH):
                sl_lo = slice(RATIO + e * TCH, RATIO + (e + 1) * TCH)
                sl_hi = slice(e * TCH, (e + 1) * TCH)
                kvg_group(kvlo, slice(0, 128), wkv_s, sl_lo, hks[e])
                kvg_group(kvhi, slice(128, 256), wkv_s, sl_hi, hks[e])
                if e == 0:
                    kvg_group(glo, slice(0, 128), wg_s, slice(0, RATIO), hh)
                kvg_group(glo, slice(0, 128), wg_s, sl_lo, hks[e])
                kvg_group(ghi, slice(128, 256), wg_s, sl_hi, hks[e])

            # ---- overlap gated pooling over own 256 windows ----
            numer = pb.tile([HD, WL], F32, tag="numer", name="numer")
            denom = pb.tile([HD, WL], F32, tag="denom", name="denom")
            for half, (g_src, kv_src, acol, n) in enumerate(
                ((glo, kvlo, slice(0, RATIO), HKV),
                 (ghi, kvhi, slice(RATIO, 2 * RATIO), TL))
            ):
                e_t = pb.tile([HD, HKV], F32, tag="poole", name=f"poole{half}")
                nc.scalar.activation(e_t[:, 0:n], g_src[:, 0:n], AF.Exp)
                nc.vector.tensor_mul(
                    e_t[:, 0:n].rearrange("d (w r) -> d w r", r=RATIO),
                    e_t[:, 0:n].rearrange("d (w r) -> d w r", r=RATIO),
                    eape_s[:, None, acol].to_broadcast(
                        [HD, n // RATIO, RATIO]))
                ea = pb.tile([HD, HKV], F32, tag="poolea", name=f"poolea{half}")
                nc.vector.tensor_mul(ea[:, 0:n], e_t[:, 0:n], kv_src[:, 0:n])
                for acc, src in ((denom, e_t), (numer, ea)):
                    s3 = src[:, 0:TL].rearrange("d (w r) -> d w r", r=RATIO)
                    nm = f"poolred{half}{1 if acc is numer else 0}"
                    ra = pb.tile([HD, WL], F32, tag="poolra", name=nm + "a")
                    nc.vector.tensor_add(ra[:], s3[:, :, 0], s3[:, :, 1])
                    rc = pb.tile([HD, WL], F32, tag="poolrc", name=nm + "c")
                    nc.vector.tensor_add(rc[:], s3[:, :, 2], s3[:, :, 3])
                    if half == 0:
                        nc.vector.tensor_add(acc[:], ra[:], rc[:])
                    else:
                        nc.vector.tensor_add(ra[:], ra[:], rc[:])
                        nc.vector.tensor_add(acc[:], acc[:], ra[:])
            rec = pb.tile([HD, WL], F32, tag="poolrec", name="poolrec")
            nc.vector.reciprocal(rec[:], denom[:])
            pool_bf = pb.tile([HD, WL], F32, tag="pool_bf", name="pool_bf")
            nc.vector.tensor_mul(pool_bf[:], numer[:], rec[:])

            # ---- AllGather pooled windows (bf16) across the batch group ----
            nc.sync.dma_start(cc_in[:], pool_bf[:])
            nc.gpsimd.collective_compute(
                "AllGather", mybir.AluOpType.bypass,
                replica_groups=[[0, 1, 2, 3], [4, 5, 6, 7]],
                ins=[cc_in[:]], outs=[cc_out[:]],
            )

            # ---- q projections for chunks 0..3 (PE cover for the AG) ----
            nc.sync.dma_start(cosq_s[RD:128, :], cosq[:])
            nc.sync.dma_start(sinq_s[RD:128, :], sinq[:])
            nc.sync.dma_start(cosk_s[RD:128, :], cosk[:])
            nc.sync.dma_start(sink_s[RD:128, :], sink[:])
            nc.sync.dma_start(wo_s[:],
                              wo.rearrange("(h p) e -> p h e", p=HD))
            for j in range(4):
                for m in range(HPC):
                    qp_unit(psP, hsl_cache[j], j, m, qp_bufs=2)

            # ---- unpack AG, rope -> kT, transpose -> V ----
            for r in range(GSZ):
                nc.sync.dma_start(pg[:, ts(r, WL)], cc_out[r])
            nc.scalar.copy(kT[0:RD, :], pg[0:RD, :])
            krb = pb.tile([128, NW], BF16, tag="krope", name="krope")
            nc.scalar.copy(krb[RD:128, :], pg[RD:128, :])
            ksw = pb.tile([128, NW], BF16, tag="kswap", name="kswap")
            nc.vector.stream_shuffle(ksw[RD:128, :], krb[RD:128, :], _PAIR_SWAP)
            kt1 = pb.tile([128, NW], BF16, tag="kt1", name="kt1")
            nc.vector.tensor_mul(kt1[RD:128, :], krb[RD:128, :],
                                 cosk_s[RD:128, :])
            kt2 = pb.tile([128, NW], BF16, tag="kt2", name="kt2")
            nc.vector.tensor_mul(kt2[RD:128, :], ksw[RD:128, :],
                                 sink_s[RD:128, :])
            nc.vector.tensor_add(kT[RD:128, :], kt1[RD:128, :], kt2[RD:128, :])
            for wb in range(NW // WCH):
                tp = psP.tile([128, 128], F32, tag="vtrans", name="vtrans",
                              bufs=2)
                nc.tensor.transpose(tp[:], pg[:, ts(wb, 128)], ident[:])
                nc.scalar.copy(v_s[:, wb, :], tp[:])

        # =============== main loop ===============
        with (
            tc.tile_pool(name="psA", bufs=1, space="PSUM") as psA,
            tc.tile_pool(name="psS", bufs=2, space="PSUM") as psS,
            tc.tile_pool(name="psAcc", bufs=2, space="PSUM") as pacc,
            tc.tile_pool(name="psDen", bufs=1, space="PSUM") as pden,
            tc.tile_pool(name="psO", bufs=2, space="PSUM") as pso,
            tc.tile_pool(name="pt", bufs=4) as ptp,
            tc.tile_pool(name="att", bufs=2) as att,
        ):
            hsl_cache[4] = load_hsl(4)

            attnT = {}         # (j, h) -> a_sb tile
            pending = None     # deferred epilogue context

            def at_stream(j, h):
                qsl = qT[h][:, ts(j, TCH)]
                den_ps = pden.tile([1, TCH], F32, tag="den", name="den")
                acc_ps = pacc.tile([128, TCH], F32, tag="acc", name="acc")
                s_tiles = {}

                def score(wc):
                    sp = psS.tile([WCH, TCH], F32, tag="s", name="s")
                    nc.tensor.matmul(sp[:], kT[:, ts(wc, WCH)], qsl,
                                     start=True, stop=True)
                    s_tiles[wc] = sp

                score(0)
                for wc in range(j + 1):
                    if wc < j:
                        score(wc + 1)
                    sp = s_tiles.pop(wc)
                    pt = ptp.tile([WCH, TCH], BF16, tag="pt", name="pt")
                    nc.scalar.activation(pt[:], sp[:], AF.Exp, scale=SCALE)
                    if wc == j:
                        ptm = ptp.tile([WCH, TCH], BF16, tag="ptm", name="ptm")
                        nc.vector.tensor_mul(ptm[:], pt[:], bandm_s[:])
                        pt = ptm
                    nc.tensor.matmul(den_ps[:], ones_w[:], pt[:],
                                     start=(wc == 0), stop=(wc == j))
                    nc.tensor.matmul(acc_ps[:], v_s[:, wc, :], pt[:],
                                     start=(wc == 0), stop=(wc == j))
                # early (ACT/DVE) part of the epilogue; PE part is deferred
                den_sb = att.tile([1, TCH], F32, tag="den_sb", name="den_sb")
                nc.scalar.activation(den_sb[:], den_ps[:], AF.Identity,
                                     bias=esink_s[0:1, h:h + 1])
                rec_sb = att.tile([1, TCH], F32, tag="rec_sb", name="rec_sb")
                nc.vector.reciprocal(rec_sb[:], den_sb[:])
                return (j, h, acc_ps, rec_sb)

            def at_epilogue(ctx):
                j, h, acc_ps, rec_sb = ctx
                rb_ps = pso.tile([128, TCH], F32, tag="o", name="o")
                nc.tensor.matmul(rb_ps[:], ones_p[:], rec_sb[:],
                                 start=True, stop=True)
                rb_sb = att.tile([128, TCH], F32, tag="rb_sb", name="rb_sb")
                nc.vector.tensor_copy(rb_sb[:], rb_ps[:])
                a_sb = att.tile([128, TCH], BF16, tag=f"attnT{h}",
                                name=f"attnT{h}")
                nc.vector.tensor_mul(a_sb[:], acc_ps[:], rb_sb[:])
                attnT[(j, h)] = a_sb

            def op_quarter(j, qh):
                for idx in range(4 * qh, 4 * qh + 4):
                    tt, e = divmod(idx, HID // TCH)
                    o_ps = pso.tile([128, TCH], F32, tag="o", name="o")
                    for h in range(HPC):
                        nc.tensor.matmul(o_ps[:],
                                         attnT[(j, h)][:, ts(tt, 128)],
                                         wo_s[:, h, ts(e, TCH)],
                                         start=(h == 0), stop=(h == HPC - 1))
                    o_sb = att.tile([128, TCH], BF16, tag="o_sb", name="o_sb")
                    if e % 2 == 0:
                        nc.scalar.copy(o_sb[:], o_ps[:])
                    else:
                        nc.vector.tensor_copy(o_sb[:], o_ps[:])
                    nc.sync.dma_start(
                        out[j * TCH + tt * 128:j * TCH + (tt + 1) * 128,
                            ts(e, TCH)], o_sb[:])

            for j in range(NCH):
                if j + 5 < NCH:
                    hsl_cache[j + 5] = load_hsl(j + 5)
                for h in range(HPC):
                    ctx = at_stream(j, h)
                    if pending is not None:
                        at_epilogue(pending)
                    pending = ctx
                    if j + 4 < NCH:
                        qp_unit(psA, hsl_cache[j + 4], j + 4, h)
                    if j >= 1:
                        op_quarter(j - 1, h)
                hsl_cache.pop(j + 4, None)
            at_epilogue(pending)
            op_quarter(NCH - 1, 0)
            op_quarter(NCH - 1, 1)
            op_quarter(NCH - 1, 2)
            op_quarter(NCH - 1, 3)


_WS_CTR = [0]


def _split_multi_waits(nc):
    """This walrus build accepts at most ONE sync wait per instruction; hoist
    extras onto same-engine NOPs placed immediately before."""
    f = nc.m.functions[0]
    for blk in f.blocks:
        insts = blk.instructions
        if not any(i.sync_info is not None and len(i.sync_info.on_wait) > 1
                   for i in insts):
            continue
        new_list = []
        for inst in insts:
            si = inst.sync_info
            if si is not None and len(si.on_wait) > 1:
                waits = list(si.on_wait)
                for w in waits[:-1]:
                    _WS_CTR[0] += 1
                    new_list.append(mybir.InstNoOp(
                        name=f"waitsplit-{_WS_CTR[0]}",
                        engine=inst.engine,
                        bass_nofuse=True,
                        sync_info=mybir.SyncInfo(on_wait=[w], on_update=[])))
                inst.sync_info = mybir.SyncInfo(
                    on_wait=[waits[-1]], on_update=list(si.on_update))
            new_list.append(inst)
        blk.instructions = new_list


# ---------------------------------------------------------------------------
# host side
# ---------------------------------------------------------------------------

def _rope_tables(positions):
    half = RD // 2
    inv_freq = 1.0 / (THETA ** (np.arange(half, dtype=np.float64) / half))
    ang = positions[None, :].astype(np.float64) * inv_freq[:, None]  # [32, L]
    cos_t = np.repeat(np.cos(ang), 2, axis=0).astype(np.float32)
    sin_t = np.repeat(np.sin(ang), 2, axis=0).astype(np.float32)
    sin_t[0::2] *= -1.0                                  # a-rows get -sin
    return cos_t, sin_t


def _prep_inputs(hidden, wq, wkv, wgate, ape, sinks, wo):
    bf = ml_dtypes.bfloat16
    cosq_t, sinq_t = _rope_tables(np.arange(S))
    cosk_t, sink_t = _rope_tables(np.arange(NW) * RATIO)
    pw, ft = np.meshgrid(np.arange(WCH), np.arange(TCH), indexing="ij")
    band = (ft >= RATIO * pw + RATIO - 1).astype(np.float32)     # [WCH, TCH]
    eape = np.empty((HD, 2 * RATIO), np.float32)
    for r in range(RATIO):
        eape[:, r] = np.exp(ape[r, :HD])
        eape[:, RATIO + r] = np.exp(ape[r, HD:])
    maps = []
    for c in range(N_CORES):
        b, g = divmod(c, GSZ)
        htc = np.ascontiguousarray(hidden[b].T).astype(bf)     # [HID, S]
        htkv = np.zeros((HID, HKV), bf)
        lo = TL * g - RATIO
        htkv[:, (RATIO if g == 0 else 0):] = htc[:, max(lo, 0):TL * (g + 1)]
        maps.append({
            "ht": htc,
            "htkv": htkv,
            "wq": np.ascontiguousarray(wq[:, g * CW:(g + 1) * CW]).astype(bf),
            "wkv": wkv.astype(bf),
            "wg": wgate.astype(bf),
            "wo": np.ascontiguousarray(wo[g * CW:(g + 1) * CW, :]).astype(bf),
            "eape": eape,
            "esink": np.exp(sinks[g * HPC:(g + 1) * HPC]).astype(
                np.float32).reshape(1, HPC),
            "gfix": np.full((HD, 1), -30000.0 if g == 0 else 0.0, np.float32),
            "cosq": cosq_t.astype(bf), "sinq": sinq_t.astype(bf),
            "cosk": cosk_t.astype(bf), "sink": sink_t.astype(bf),
            "bandm": band.astype(bf),
        })
    return maps


_RUNNER_CACHE = {}


def _get_runner(n_reps: int = 1):
    if n_reps in _RUNNER_CACHE:
        return _RUNNER_CACHE[n_reps]
    import jax
    from jax.sharding import Mesh, PartitionSpec
    from jax.experimental.shard_map import shard_map
    from concourse.bass2jax import (_bass_exec_p, install_neuronx_cc_hook,
                                    partition_id_tensor)

    nc = _build_nc(n_reps)
    install_neuronx_cc_hook()
    partition_name = nc.partition_id_tensor.name if nc.partition_id_tensor else None
    in_names, out_names, out_avals, zero_outs = [], [], [], []
    for alloc in nc.m.functions[0].allocations:
        if not isinstance(alloc, mybir.MemoryLocationSet):
            continue
        name = alloc.memorylocations[0].name
        if alloc.kind == "ExternalInput":
            if name != partition_name:
                in_names.append(name)
        elif alloc.kind == "ExternalOutput":
            out_names.append(name)
            shape = tuple(alloc.tensor_shape)
            dtype = mybir.dt.np(alloc.dtype)
            out_avals.append(jax.core.ShapedArray(shape, dtype))
            zero_outs.append(np.zeros(shape, dtype))
    n_params = len(in_names)
    all_in_names = list(in_names) + out_names
    if partition_name is not None:
        all_in_names.append(partition_name)

    def _kernel_body(*args):
        operands = list(args)
        if partition_name is not None:
            operands.append(partition_id_tensor())
        outs = _bass_exec_p.bind(
            *operands,
            out_avals=tuple(out_avals),
            in_names=tuple(all_in_names),
            out_names=tuple(out_names),
            lowering_input_output_aliases=(),
            sim_require_finite=True,
            sim_require_nnan=True,
            nc=nc,
        )
        return tuple(outs)

    devices = jax.devices()[:N_CORES]
    mesh = Mesh(np.asarray(devices), ("core",))
    spec = PartitionSpec("core")
    fn = jax.jit(shard_map(
        _kernel_body, mesh=mesh,
        in_specs=(spec,) * (n_params + len(out_names)),
        out_specs=(spec,) * len(out_names), check_rep=False))
    runner = (fn, in_names, out_names, zero_outs, mesh)
    _RUNNER_CACHE[n_reps] = runner
    return runner


def _run_core_maps(maps, n_reps: int = 1):
    import jax
    from jax.sharding import NamedSharding, PartitionSpec
    fn, in_names, out_names, zero_outs, mesh = _get_runner(n_reps)
    sh = NamedSharding(mesh, PartitionSpec("core"))
    args = [jax.device_put(
        np.concatenate([np.asarray(m[name]) for m in maps], axis=0), sh)
        for name in in_names]
    for z in zero_outs:
        args.append(jax.device_put(
            np.zeros((N_CORES * z.shape[0], *z.shape[1:]), z.dtype), sh))
    res = fn(*args)
    jax.block_until_ready(res)
    return np.asarray(res[0]).astype(np.float32).reshape(N_CORES, S, HID)


def kernel(hidden, wq, wkv, wgate, ape, sinks, wo,
           ratio=RATIO, head_dim=HD, rope_head_dim=RD, num_heads=NH):
    hidden = np.asarray(hidden, np.float32)
    maps = _prep_inputs(hidden, np.asarray(wq, np.float32),
                        np.asarray(wkv, np.float32),
                        np.asarray(wgate, np.float32),
                        np.asarray(ape, np.float32),
                        np.asarray(sinks, np.float32),
                        np.asarray(wo, np.float32))
    partials = _run_core_maps(maps)
    out = np.empty((B, S, HID), np.float32)
    for b in range(B):
        out[b] = partials[b * HPC:(b + 1) * HPC].astype(np.float64).sum(
            axis=0).astype(np.float32)
    return out


# revision 4
# speedup vs baseline: 1.3878x; 1.0278x over previous
"""DeepseekV4-style sparse attention on 8 Trainium2 cores (Bass/Tile), v2.

Sharding: data-parallel over batch (2) x tensor-parallel over heads (16 -> 4
groups of 4).  Core c handles batch c//4 and heads [4*(c%4), 4*(c%4)+4).
NEW in v2: the kv/gate projections + gated pooling are sharded over the 4
cores of each batch group (each core computes its own S/4 token slice plus a
4-token halo, pools its 256 windows, and the groups AllGather the pooled
windows in bf16).  Per-core partial outputs (attn_heads @ wo_rows) are
summed on the host.

Issue-order pipeline (PE never idles for long):
  prologue: kv/gate shard projections -> pooling -> AllGather (on TOPSP/SDMA,
            overlapped) -> q projections for chunks 0,1 -> kT rope + V
            transposes
  main loop over q-chunks j, heads h:
            score/exp/den/PV stream for (j,h), then the *deferred* epilogue
            of the previous stream, then one q-projection unit for chunk j+2
            and one out-projection quarter for chunk j-1 as PE filler.
PSUM budget (8 banks): qproj 1 + scores 2 + acc 2 + den 1 + out/rb 2.
"""

import numpy as np
import ml_dtypes

import concourse.bass as bass
import concourse.mybir as mybir
import concourse.tile as tile
from concourse.bass import ts
from concourse.masks import make_identity

F32 = mybir.dt.float32
BF16 = mybir.dt.bfloat16
AF = mybir.ActivationFunctionType

# Problem constants (hardcoded per the harness contract).
B, S, HID, NH, HD, RD, RATIO = 2, 4096, 2048, 16, 128, 64, 4
THETA = 10000.0
NW = S // RATIO              # 1024 pooled windows
N_CORES = 8
GSZ = 4                      # cores per batch group (kv/gate shard factor)
TL = S // GSZ                # own token slice (1024)
WL = NW // GSZ               # own window slice (256)
HKV = TL + RATIO             # htkv cols: 4-token halo + own slice
HPC = 4                      # heads per core
CW = HPC * HD                # per-core q/wo width (512)
TCH = 512                    # t-chunk size
NCH = S // TCH               # 8 t-chunks
WCH = 128                    # w-chunk size
KCH = HID // 128             # 16 contraction chunks
SCALE = HD ** -0.5

_PAIR_SWAP = [i ^ 1 for i in range(32)]


def _build_nc(n_reps: int = 1, split_waits: bool = True):
    nc = bass.Bass(num_devices=N_CORES)
    dp = nc.declare_dram_parameter
    ht = dp("ht", [HID, S], BF16, isOutput=False)
    htkv = dp("htkv", [HID, HKV], BF16, isOutput=False)
    wq = dp("wq", [HID, CW], BF16, isOutput=False)
    wkv = dp("wkv", [HID, 2 * HD], BF16, isOutput=False)
    wg = dp("wg", [HID, 2 * HD], BF16, isOutput=False)
    wo = dp("wo", [CW, HID], BF16, isOutput=False)
    eape = dp("eape", [HD, 2 * RATIO], F32, isOutput=False)
    esink = dp("esink", [1, HPC], F32, isOutput=False)
    gfix = dp("gfix", [HD, 1], F32, isOutput=False)
    cosq = dp("cosq", [RD, S], BF16, isOutput=False)
    sinq = dp("sinq", [RD, S], BF16, isOutput=False)
    cosk = dp("cosk", [RD, NW], BF16, isOutput=False)
    sink = dp("sink", [RD, NW], BF16, isOutput=False)
    bandm = dp("bandm", [WCH, TCH], BF16, isOutput=False)
    out = dp("out", [S, HID], BF16, isOutput=True)
    args = (ht, htkv, wq, wkv, wg, wo, eape, esink, gfix,
            cosq, sinq, cosk, sink, bandm, out)

    # collectives cannot sit inside a hardware For_i loop, so timing builds
    # replicate the whole body as straight-line code instead
    with tile.TileContext(nc) as tc:
        for r in range(n_reps):
            _body(nc, tc, *args, rep=r)
    if split_waits:
        _split_multi_waits(nc)
    return nc


def _body(nc, tc, ht, htkv, wq, wkv, wg, wo, eape, esink, gfix,
          cosq, sinq, cosk, sink, bandm, out, rep=0):
    cc_in = nc.dram_tensor(f"cc_in{rep}", [128, WL], F32)
    cc_out = nc.dram_tensor(f"cc_out{rep}", [GSZ, 128, WL], F32)

    with (
        tc.tile_pool(name="persist", bufs=1) as pp,
        tc.tile_pool(name="hslab", bufs=2) as hs,
    ):
        # ---- persistent SBUF state ----
        qT = [pp.tile([128, S], BF16, tag=f"qT{m}", name=f"qT{m}")
              for m in range(HPC)]
        kvlo = pp.tile([HD, HKV], BF16, tag="kvlo", name="kvlo")
        kvhi = pp.tile([HD, TL], BF16, tag="kvhi", name="kvhi")
        glo = pp.tile([HD, HKV], BF16, tag="glo", name="glo")
        ghi = pp.tile([HD, TL], BF16, tag="ghi", name="ghi")
        cosq_s = pp.tile([128, S], BF16, tag="cosq", name="cosq")
        sinq_s = pp.tile([128, S], BF16, tag="sinq", name="sinq")
        cosk_s = pp.tile([128, NW], BF16, tag="cosk", name="cosk")
        sink_s = pp.tile([128, NW], BF16, tag="sink", name="sink")
        eape_s = pp.tile([HD, 2 * RATIO], F32, tag="eape", name="eape")
        esink_s = pp.tile([1, HPC], F32, tag="esink", name="esink")
        gfix_s = pp.tile([HD, 1], F32, tag="gfix", name="gfix")
        bandm_s = pp.tile([WCH, TCH], BF16, tag="bandm", name="bandm")
        wo_s = pp.tile([HD, HPC, HID], BF16, tag="wo", name="wo")
        wq_s = pp.tile([128, KCH, CW], BF16, tag="wq", name="wq")
        wkv_s = pp.tile([128, KCH, 2 * HD], BF16, tag="wkv", name="wkv")
        wg_s = pp.tile([128, KCH, 2 * HD], BF16, tag="wg", name="wg")
        ones_w = pp.tile([WCH, 1], BF16, tag="ones_w", name="ones_w")
        ones_p = pp.tile([1, HD], F32, tag="ones_p", name="ones_p")
        kT = pp.tile([HD, NW], BF16, tag="kT", name="kT")
        v_s = pp.tile([WCH, NW // WCH, HD], BF16, tag="v", name="v")
        pg = pp.tile([128, NW], F32, tag="pg", name="pg")
        ident = pp.tile([128, 128], F32, tag="ident", name="ident")

        # DMA issue order is queue order: kv/gate weights + small tables
        # first so the PE can start within ~6us; bulky tables later.
        wkv_r = wkv.rearrange("(k p) c -> p k c", p=128)
        wg_r = wg.rearrange("(k p) c -> p k c", p=128)
        H = KCH // 2
        nc.sync.dma_start(wkv_s[:, 0:H], wkv_r[:, 0:H])
        nc.sync.dma_start(wkv_s[:, H:KCH], wkv_r[:, H:KCH])
        nc.sync.dma_start(eape_s[:], eape[:])
        nc.sync.dma_start(esink_s[:], esink[:])
        nc.sync.dma_start(gfix_s[:], gfix[:])
        nc.sync.dma_start(bandm_s[:], bandm[:])
        nc.vector.memset(ones_w[:], 1.0)
        nc.vector.memset(ones_p[:], 1.0)
        make_identity(nc, ident[:])

        def load_hsl(j):
            hsl = hs.tile([128, KCH, TCH], BF16, tag="hslab", name="hslab",
                          bufs=2)
            nc.sync.dma_start(
                hsl[:], ht[:, ts(j, TCH)].rearrange("(k p) t -> p k t", p=128))
            return hsl

        def qp_unit(psA, hsl, j, m, qp_bufs=1):
            """q projection for head m, chunk j, with fused interleaved RoPE."""
            tsl = ts(j, TCH)
            ps = psA.tile([128, TCH], F32, tag="psA", name="psA",
                          bufs=qp_bufs)
            for k in range(KCH):
                nc.tensor.matmul(ps[:], wq_s[:, k, ts(m, 128)], hsl[:, k, :],
                                 start=(k == 0), stop=(k == KCH - 1))
            nc.scalar.copy(qT[m][0:RD, tsl], ps[0:RD, :])
            rb = hs.tile([128, TCH], BF16, tag="ropebuf", name="ropebuf")
            nc.scalar.copy(rb[RD:128, :], ps[RD:128, :])
            sw = hs.tile([128, TCH], BF16, tag="ropeswap", name="ropeswap")
            nc.vector.stream_shuffle(sw[RD:128, :], rb[RD:128, :], _PAIR_SWAP)
            t1 = hs.tile([128, TCH], BF16, tag="ropet1", name="ropet1")
            nc.vector.tensor_mul(t1[RD:128, :], rb[RD:128, :],
                                 cosq_s[RD:128, tsl])
            t2 = hs.tile([128, TCH], BF16, tag="ropet2", name="ropet2")
            nc.vector.tensor_mul(t2[RD:128, :], sw[RD:128, :],
                                 sinq_s[RD:128, tsl])
            nc.vector.tensor_add(qT[m][RD:128, tsl], t1[RD:128, :],
                                 t2[RD:128, :])

        # =============== prologue scope ===============
        with (
            tc.tile_pool(name="psP", bufs=3, space="PSUM") as psP,
            tc.tile_pool(name="poolb", bufs=1) as pb,
        ):
            # ---- kv/gate shard projections (own slice + 4-token halo) ----
            hh = pb.tile([128, KCH, RATIO], BF16, tag="hhalo", name="hhalo")
            nc.sync.dma_start(
                hh[:], htkv[:, 0:RATIO].rearrange("(k p) t -> p k t", p=128))
            hks = []
            for e in range(TL // TCH):
                hk = hs.tile([128, KCH, TCH], BF16, tag="hslab", name="hslab",
                             bufs=2)
                hkr = htkv[:, RATIO + e * TCH: RATIO + (e + 1) * TCH
                           ].rearrange("(k p) t -> p k t", p=128)
                nc.sync.dma_start(hk[:, 0:H], hkr[:, 0:H])
                if e == 0:
                    nc.sync.dma_start(wg_s[:, 0:H], wg_r[:, 0:H])
                    nc.sync.dma_start(wg_s[:, H:KCH], wg_r[:, H:KCH])
                nc.sync.dma_start(hk[:, H:KCH], hkr[:, H:KCH])
                hks.append(hk)
            hsl_cache = {}
            for j in range(2):
                hsl_cache[j] = load_hsl(j)
            nc.sync.dma_start(wq_s[:], wq.rearrange("(k p) c -> p k c", p=128))
            for j in range(2, 4):
                hsl_cache[j] = load_hsl(j)

            def kvg_group(dst, col, wsrc, dslice, rhs):
                ps = psP.tile([128, TCH], F32, tag="psP", name="psP")
                n = rhs.shape[-1]
                for k in range(KCH):
                    nc.tensor.matmul(ps[:, 0:n], wsrc[:, k, col], rhs[:, k, :],
                                     start=(k == 0), stop=(k == KCH - 1))
                if dst is glo and dslice.start == 0:
                    nc.scalar.activation(dst[:, dslice], ps[:, 0:n],
                                         AF.Identity, bias=gfix_s[:, 0:1])
                else:
                    nc.scalar.copy(dst[:, dslice], ps[:, 0:n])

            # halo kv first (only needs wkv + the tiny halo slab)
            kvg_group(kvlo, slice(0, 128), wkv_s, slice(0, RATIO), hh)
            for e in range(TL // TCH):
                sl_lo = slice(RATIO + e * TCH, RATIO + (e + 1) * TCH)
                sl_hi = slice(e * TCH, (e + 1) * TCH)
                kvg_group(kvlo, slice(0, 128), wkv_s, sl_lo, hks[e])
                kvg_group(kvhi, slice(128, 256), wkv_s, sl_hi, hks[e])
                if e == 0:
                    kvg_group(glo, slice(0, 128), wg_s, slice(0, RATIO), hh)
                kvg_group(glo, slice(0, 128), wg_s, sl_lo, hks[e])
                kvg_group(ghi, slice(128, 256), wg_s, sl_hi, hks[e])

            # ---- overlap gated pooling over own 256 windows ----
            numer = pb.tile([HD, WL], F32, tag="numer", name="numer")
            denom = pb.tile([HD, WL], F32, tag="denom", name="denom")
            for half, (g_src, kv_src, acol, n) in enumerate(
                ((glo, kvlo, slice(0, RATIO), HKV),
                 (ghi, kvhi, slice(RATIO, 2 * RATIO), TL))
            ):
                e_t = pb.tile([HD, HKV], F32, tag="poole", name=f"poole{half}")
                nc.scalar.activation(e_t[:, 0:n], g_src[:, 0:n], AF.Exp)
                nc.vector.tensor_mul(
                    e_t[:, 0:n].rearrange("d (w r) -> d w r", r=RATIO),
                    e_t[:, 0:n].rearrange("d (w r) -> d w r", r=RATIO),
                    eape_s[:, None, acol].to_broadcast(
                        [HD, n // RATIO, RATIO]))
                ea = pb.tile([HD, HKV], F32, tag="poolea", name=f"poolea{half}")
                nc.vector.tensor_mul(ea[:, 0:n], e_t[:, 0:n], kv_src[:, 0:n])
                for acc, src in ((denom, e_t), (numer, ea)):
                    s3 = src[:, 0:TL].rearrange("d (w r) -> d w r", r=RATIO)
                    nm = f"poolred{half}{1 if acc is numer else 0}"
                    ra = pb.tile([HD, WL], F32, tag="poolra", name=nm + "a")
                    nc.vector.tensor_add(ra[:], s3[:, :, 0], s3[:, :, 1])
                    rc = pb.tile([HD, WL], F32, tag="poolrc", name=nm + "c")
                    nc.vector.tensor_add(rc[:], s3[:, :, 2], s3[:, :, 3])
                    if half == 0:
                        nc.vector.tensor_add(acc[:], ra[:], rc[:])
                    else:
                        nc.vector.tensor_add(ra[:], ra[:], rc[:])
                        nc.vector.tensor_add(acc[:], acc[:], ra[:])
            rec = pb.tile([HD, WL], F32, tag="poolrec", name="poolrec")
            nc.vector.reciprocal(rec[:], denom[:])
            pool_bf = pb.tile([HD, WL], F32, tag="pool_bf", name="pool_bf")
            nc.vector.tensor_mul(pool_bf[:], numer[:], rec[:])

            # ---- AllGather pooled windows (bf16) across the batch group ----
            nc.sync.dma_start(cc_in[:], pool_bf[:])
            nc.gpsimd.collective_compute(
                "AllGather", mybir.AluOpType.bypass,
                replica_groups=[[0, 1, 2, 3], [4, 5, 6, 7]],
                ins=[cc_in[:]], outs=[cc_out[:]],
            )

            # ---- q projections for chunks 0..3 (PE cover for the AG) ----
            nc.sync.dma_start(cosq_s[RD:128, :], cosq[:])
            nc.sync.dma_start(sinq_s[RD:128, :], sinq[:])
            nc.sync.dma_start(cosk_s[RD:128, :], cosk[:])
            nc.sync.dma_start(sink_s[RD:128, :], sink[:])
            nc.sync.dma_start(wo_s[:],
                              wo.rearrange("(h p) e -> p h e", p=HD))
            for j in range(4):
                for m in range(HPC):
                    qp_unit(psP, hsl_cache[j], j, m, qp_bufs=2)
            hsl_cache[4] = load_hsl(4)
            for m in range(HPC):
                qp_unit(psP, hsl_cache[4], 4, m, qp_bufs=2)

            # ---- unpack AG, rope -> kT, transpose -> V ----
            for r in range(GSZ):
                nc.sync.dma_start(pg[:, ts(r, WL)], cc_out[r])
            nc.scalar.copy(kT[0:RD, :], pg[0:RD, :])
            krb = pb.tile([128, NW], BF16, tag="krope", name="krope")
            nc.scalar.copy(krb[RD:128, :], pg[RD:128, :])
            ksw = pb.tile([128, NW], BF16, tag="kswap", name="kswap")
            nc.vector.stream_shuffle(ksw[RD:128, :], krb[RD:128, :], _PAIR_SWAP)
            kt1 = pb.tile([128, NW], BF16, tag="kt1", name="kt1")
            nc.vector.tensor_mul(kt1[RD:128, :], krb[RD:128, :],
                                 cosk_s[RD:128, :])
            kt2 = pb.tile([128, NW], BF16, tag="kt2", name="kt2")
            nc.vector.tensor_mul(kt2[RD:128, :], ksw[RD:128, :],
                                 sink_s[RD:128, :])
            nc.vector.tensor_add(kT[RD:128, :], kt1[RD:128, :], kt2[RD:128, :])
            for wb in range(NW // WCH):
                tp = psP.tile([128, 128], F32, tag="vtrans", name="vtrans",
                              bufs=2)
                nc.tensor.transpose(tp[:], pg[:, ts(wb, 128)], ident[:])
                nc.scalar.copy(v_s[:, wb, :], tp[:])

        # =============== main loop ===============
        with (
            tc.tile_pool(name="psA", bufs=1, space="PSUM") as psA,
            tc.tile_pool(name="psS", bufs=2, space="PSUM") as psS,
            tc.tile_pool(name="psAcc", bufs=2, space="PSUM") as pacc,
            tc.tile_pool(name="psDen", bufs=1, space="PSUM") as pden,
            tc.tile_pool(name="psO", bufs=2, space="PSUM") as pso,
            tc.tile_pool(name="pt", bufs=4) as ptp,
            tc.tile_pool(name="att", bufs=2) as att,
        ):
            hsl_cache[5] = load_hsl(5)

            attnT = {}         # (j, h) -> a_sb tile
            pending = None     # deferred epilogue context

            def at_stream(j, h, fillers=()):
                fillers = list(fillers)
                qsl = qT[h][:, ts(j, TCH)]
                den_ps = pden.tile([1, TCH], F32, tag="den", name="den")
                acc_ps = pacc.tile([128, TCH], F32, tag="acc", name="acc")
                s_tiles = {}

                def score(wc):
                    sp = psS.tile([WCH, TCH], F32, tag="s", name="s")
                    nc.tensor.matmul(sp[:], kT[:, ts(wc, WCH)], qsl,
                                     start=True, stop=True)
                    s_tiles[wc] = sp

                # diagonal w-chunk first: its DVE mask-multiply runs while
                # the DVE queue is still empty, not behind filler evacuations
                wcs = [j] + list(range(j))
                score(wcs[0])
                for i, wc in enumerate(wcs):
                    if i + 1 < len(wcs):
                        score(wcs[i + 1])
                    sp = s_tiles.pop(wc)
                    pt = ptp.tile([WCH, TCH], BF16, tag="pt", name="pt")
                    nc.scalar.activation(pt[:], sp[:], AF.Exp, scale=SCALE)
                    if wc == j:
                        ptm = ptp.tile([WCH, TCH], BF16, tag="ptm", name="ptm")
                        nc.vector.tensor_mul(ptm[:], pt[:], bandm_s[:])
                        pt = ptm
                    nc.tensor.matmul(den_ps[:], ones_w[:], pt[:],
                                     start=(i == 0), stop=(i == len(wcs) - 1))
                    nc.tensor.matmul(acc_ps[:], v_s[:, wc, :], pt[:],
                                     start=(i == 0), stop=(i == len(wcs) - 1))
                    if i >= 1 and fillers:
                        fillers.pop(0)()
                for f in fillers:
                    f()
                # early (ACT/DVE) part of the epilogue; PE part is deferred
                den_sb = att.tile([1, TCH], F32, tag="den_sb", name="den_sb")
                nc.scalar.activation(den_sb[:], den_ps[:], AF.Identity,
                                     bias=esink_s[0:1, h:h + 1])
                rec_sb = att.tile([1, TCH], F32, tag="rec_sb", name="rec_sb")
                nc.vector.reciprocal(rec_sb[:], den_sb[:])
                return (j, h, acc_ps, rec_sb)

            def at_epilogue(ctx):
                j, h, acc_ps, rec_sb = ctx
                rb_ps = pso.tile([128, TCH], F32, tag="o", name="o")
                nc.tensor.matmul(rb_ps[:], ones_p[:], rec_sb[:],
                                 start=True, stop=True)
                rb_sb = att.tile([128, TCH], F32, tag="rb_sb", name="rb_sb")
                nc.vector.tensor_copy(rb_sb[:], rb_ps[:])
                a_sb = att.tile([128, TCH], BF16, tag=f"attnT{h}",
                                name=f"attnT{h}")
                nc.vector.tensor_mul(a_sb[:], acc_ps[:], rb_sb[:])
                attnT[(j, h)] = a_sb

            def op_group(j, idx):
                tt, e = divmod(idx, HID // TCH)
                o_ps = pso.tile([128, TCH], F32, tag="o", name="o")
                for h in range(HPC):
                    nc.tensor.matmul(o_ps[:],
                                     attnT[(j, h)][:, ts(tt, 128)],
                                     wo_s[:, h, ts(e, TCH)],
                                     start=(h == 0), stop=(h == HPC - 1))
                o_sb = att.tile([128, TCH], BF16, tag="o_sb", name="o_sb")
                if e % 2 == 0:
                    nc.scalar.copy(o_sb[:], o_ps[:])
                else:
                    nc.vector.tensor_copy(o_sb[:], o_ps[:])
                nc.sync.dma_start(
                    out[j * TCH + tt * 128:j * TCH + (tt + 1) * 128,
                        ts(e, TCH)], o_sb[:])

            def op_quarter(j, qh):
                for idx in range(4 * qh, 4 * qh + 4):
                    op_group(j, idx)

            qp_next = 5 * HPC   # flat index of next q-proj unit (chunk*4+m)
            for j in range(NCH):
                if j + 6 < NCH:
                    hsl_cache[j + 6] = load_hsl(j + 6)
                for h in range(HPC):
                    fillers = []
                    if pending is not None:
                        fillers.append(
                            lambda ctx=pending: at_epilogue(ctx))
                    if j >= 1:
                        fillers += [
                            (lambda jj=j - 1, ii=i: op_group(jj, ii))
                            for i in range(4 * h, 4 * h + 4)]
                    pending = at_stream(j, h, fillers)
                    if qp_next < NCH * HPC:
                        j2, m = divmod(qp_next, HPC)
                        qp_unit(psA, hsl_cache[j2], j2, m)
                        qp_next += 1
                for j2 in list(hsl_cache):
                    if (j2 + 1) * HPC <= qp_next:
                        hsl_cache.pop(j2)
            at_epilogue(pending)
            op_quarter(NCH - 1, 0)
            op_quarter(NCH - 1, 1)
            op_quarter(NCH - 1, 2)
            op_quarter(NCH - 1, 3)


_WS_CTR = [0]


def _split_multi_waits(nc):
    """This walrus build accepts at most ONE sync wait per instruction; hoist
    extras onto same-engine NOPs placed immediately before."""
    f = nc.m.functions[0]
    for blk in f.blocks:
        insts = blk.instructions
        if not any(i.sync_info is not None and len(i.sync_info.on_wait) > 1
                   for i in insts):
            continue
        new_list = []
        for inst in insts:
            si = inst.sync_info
            if si is not None and len(si.on_wait) > 1:
                waits = list(si.on_wait)
                for w in waits[:-1]:
                    _WS_CTR[0] += 1
                    new_list.append(mybir.InstNoOp(
                        name=f"waitsplit-{_WS_CTR[0]}",
                        engine=inst.engine,
                        bass_nofuse=True,
                        sync_info=mybir.SyncInfo(on_wait=[w], on_update=[])))
                inst.sync_info = mybir.SyncInfo(
                    on_wait=[waits[-1]], on_update=list(si.on_update))
            new_list.append(inst)
        blk.instructions = new_list


# ---------------------------------------------------------------------------
# host side
# ---------------------------------------------------------------------------

def _rope_tables(positions):
    half = RD // 2
    inv_freq = 1.0 / (THETA ** (np.arange(half, dtype=np.float64) / half))
    ang = positions[None, :].astype(np.float64) * inv_freq[:, None]  # [32, L]
    cos_t = np.repeat(np.cos(ang), 2, axis=0).astype(np.float32)
    sin_t = np.repeat(np.sin(ang), 2, axis=0).astype(np.float32)
    sin_t[0::2] *= -1.0                                  # a-rows get -sin
    return cos_t, sin_t


def _prep_inputs(hidden, wq, wkv, wgate, ape, sinks, wo):
    bf = ml_dtypes.bfloat16
    cosq_t, sinq_t = _rope_tables(np.arange(S))
    cosk_t, sink_t = _rope_tables(np.arange(NW) * RATIO)
    pw, ft = np.meshgrid(np.arange(WCH), np.arange(TCH), indexing="ij")
    band = (ft >= RATIO * pw + RATIO - 1).astype(np.float32)     # [WCH, TCH]
    eape = np.empty((HD, 2 * RATIO), np.float32)
    for r in range(RATIO):
        eape[:, r] = np.exp(ape[r, :HD])
        eape[:, RATIO + r] = np.exp(ape[r, HD:])
    maps = []
    for c in range(N_CORES):
        b, g = divmod(c, GSZ)
        htc = np.ascontiguousarray(hidden[b].T).astype(bf)     # [HID, S]
        htkv = np.zeros((HID, HKV), bf)
        lo = TL * g - RATIO
        htkv[:, (RATIO if g == 0 else 0):] = htc[:, max(lo, 0):TL * (g + 1)]
        maps.append({
            "ht": htc,
            "htkv": htkv,
            "wq": np.ascontiguousarray(wq[:, g * CW:(g + 1) * CW]).astype(bf),
            "wkv": wkv.astype(bf),
            "wg": wgate.astype(bf),
            "wo": np.ascontiguousarray(wo[g * CW:(g + 1) * CW, :]).astype(bf),
            "eape": eape,
            "esink": np.exp(sinks[g * HPC:(g + 1) * HPC]).astype(
                np.float32).reshape(1, HPC),
            "gfix": np.full((HD, 1), -30000.0 if g == 0 else 0.0, np.float32),
            "cosq": cosq_t.astype(bf), "sinq": sinq_t.astype(bf),
            "cosk": cosk_t.astype(bf), "sink": sink_t.astype(bf),
            "bandm": band.astype(bf),
        })
    return maps


_RUNNER_CACHE = {}


def _get_runner(n_reps: int = 1):
    if n_reps in _RUNNER_CACHE:
        return _RUNNER_CACHE[n_reps]
    import jax
    from jax.sharding import Mesh, PartitionSpec
    from jax.experimental.shard_map import shard_map
    from concourse.bass2jax import (_bass_exec_p, install_neuronx_cc_hook,
                                    partition_id_tensor)

    nc = _build_nc(n_reps)
    install_neuronx_cc_hook()
    partition_name = nc.partition_id_tensor.name if nc.partition_id_tensor else None
    in_names, out_names, out_avals, zero_outs = [], [], [], []
    for alloc in nc.m.functions[0].allocations:
        if not isinstance(alloc, mybir.MemoryLocationSet):
            continue
        name = alloc.memorylocations[0].name
        if alloc.kind == "ExternalInput":
            if name != partition_name:
                in_names.append(name)
        elif alloc.kind == "ExternalOutput":
            out_names.append(name)
            shape = tuple(alloc.tensor_shape)
            dtype = mybir.dt.np(alloc.dtype)
            out_avals.append(jax.core.ShapedArray(shape, dtype))
            zero_outs.append(np.zeros(shape, dtype))
    n_params = len(in_names)
    all_in_names = list(in_names) + out_names
    if partition_name is not None:
        all_in_names.append(partition_name)

    def _kernel_body(*args):
        operands = list(args)
        if partition_name is not None:
            operands.append(partition_id_tensor())
        outs = _bass_exec_p.bind(
            *operands,
            out_avals=tuple(out_avals),
            in_names=tuple(all_in_names),
            out_names=tuple(out_names),
            lowering_input_output_aliases=(),
            sim_require_finite=True,
            sim_require_nnan=True,
            nc=nc,
        )
        return tuple(outs)

    devices = jax.devices()[:N_CORES]
    mesh = Mesh(np.asarray(devices), ("core",))
    spec = PartitionSpec("core")
    fn = jax.jit(shard_map(
        _kernel_body, mesh=mesh,
        in_specs=(spec,) * (n_params + len(out_names)),
        out_specs=(spec,) * len(out_names), check_rep=False))
    runner = (fn, in_names, out_names, zero_outs, mesh)
    _RUNNER_CACHE[n_reps] = runner
    return runner


def _run_core_maps(maps, n_reps: int = 1):
    import jax
    from jax.sharding import NamedSharding, PartitionSpec
    fn, in_names, out_names, zero_outs, mesh = _get_runner(n_reps)
    sh = NamedSharding(mesh, PartitionSpec("core"))
    args = [jax.device_put(
        np.concatenate([np.asarray(m[name]) for m in maps], axis=0), sh)
        for name in in_names]
    for z in zero_outs:
        args.append(jax.device_put(
            np.zeros((N_CORES * z.shape[0], *z.shape[1:]), z.dtype), sh))
    res = fn(*args)
    jax.block_until_ready(res)
    return np.asarray(res[0]).astype(np.float32).reshape(N_CORES, S, HID)


def kernel(hidden, wq, wkv, wgate, ape, sinks, wo,
           ratio=RATIO, head_dim=HD, rope_head_dim=RD, num_heads=NH):
    hidden = np.asarray(hidden, np.float32)
    maps = _prep_inputs(hidden, np.asarray(wq, np.float32),
                        np.asarray(wkv, np.float32),
                        np.asarray(wgate, np.float32),
                        np.asarray(ape, np.float32),
                        np.asarray(sinks, np.float32),
                        np.asarray(wo, np.float32))
    partials = _run_core_maps(maps)
    out = np.empty((B, S, HID), np.float32)
    for b in range(B):
        out[b] = partials[b * HPC:(b + 1) * HPC].astype(np.float64).sum(
            axis=0).astype(np.float32)
    return out


# revision 5
# speedup vs baseline: 1.4125x; 1.0178x over previous
"""DeepseekV4-style sparse attention on 8 Trainium2 cores (Bass/Tile), v2.

Sharding: data-parallel over batch (2) x tensor-parallel over heads (16 -> 4
groups of 4).  Core c handles batch c//4 and heads [4*(c%4), 4*(c%4)+4).
NEW in v2: the kv/gate projections + gated pooling are sharded over the 4
cores of each batch group (each core computes its own S/4 token slice plus a
4-token halo, pools its 256 windows, and the groups AllGather the pooled
windows in bf16).  Per-core partial outputs (attn_heads @ wo_rows) are
summed on the host.

Issue-order pipeline (PE never idles for long):
  prologue: kv/gate shard projections -> pooling -> AllGather (on TOPSP/SDMA,
            overlapped) -> q projections for chunks 0,1 -> kT rope + V
            transposes
  main loop over q-chunks j, heads h:
            score/exp/den/PV stream for (j,h), then the *deferred* epilogue
            of the previous stream, then one q-projection unit for chunk j+2
            and one out-projection quarter for chunk j-1 as PE filler.
PSUM budget (8 banks): qproj 1 + scores 2 + acc 2 + den 1 + out/rb 2.
"""

import numpy as np
import ml_dtypes

import concourse.bass as bass
import concourse.mybir as mybir
import concourse.tile as tile
from concourse.bass import ts
from concourse.masks import make_identity

F32 = mybir.dt.float32
BF16 = mybir.dt.bfloat16
AF = mybir.ActivationFunctionType

# Problem constants (hardcoded per the harness contract).
B, S, HID, NH, HD, RD, RATIO = 2, 4096, 2048, 16, 128, 64, 4
THETA = 10000.0
NW = S // RATIO              # 1024 pooled windows
N_CORES = 8
GSZ = 4                      # cores per batch group (kv/gate shard factor)
TL = S // GSZ                # own token slice (1024)
WL = NW // GSZ               # own window slice (256)
HKV = TL + RATIO             # htkv cols: 4-token halo + own slice
HPC = 4                      # heads per core
CW = HPC * HD                # per-core q/wo width (512)
TCH = 512                    # t-chunk size
NCH = S // TCH               # 8 t-chunks
WCH = 128                    # w-chunk size
KCH = HID // 128             # 16 contraction chunks
SCALE = HD ** -0.5

_PAIR_SWAP = [i ^ 1 for i in range(32)]


def _build_nc(n_reps: int = 1, split_waits: bool = True):
    nc = bass.Bass(num_devices=N_CORES)
    dp = nc.declare_dram_parameter
    ht = dp("ht", [HID, S], BF16, isOutput=False)
    htkv = dp("htkv", [HID, HKV], BF16, isOutput=False)
    wq = dp("wq", [HID, CW], BF16, isOutput=False)
    wkv = dp("wkv", [HID, 2 * HD], BF16, isOutput=False)
    wg = dp("wg", [HID, 2 * HD], BF16, isOutput=False)
    wo = dp("wo", [CW, HID], BF16, isOutput=False)
    eape = dp("eape", [HD, 2 * RATIO], F32, isOutput=False)
    esink = dp("esink", [1, HPC], F32, isOutput=False)
    gfix = dp("gfix", [HD, 1], F32, isOutput=False)
    cosq = dp("cosq", [RD, S], BF16, isOutput=False)
    sinq = dp("sinq", [RD, S], BF16, isOutput=False)
    cosk = dp("cosk", [RD, NW], BF16, isOutput=False)
    sink = dp("sink", [RD, NW], BF16, isOutput=False)
    bandm = dp("bandm", [WCH, TCH], BF16, isOutput=False)
    out = dp("out", [S, HID], BF16, isOutput=True)
    args = (ht, htkv, wq, wkv, wg, wo, eape, esink, gfix,
            cosq, sinq, cosk, sink, bandm, out)

    # collectives cannot sit inside a hardware For_i loop, so timing builds
    # replicate the whole body as straight-line code instead
    with tile.TileContext(nc) as tc:
        for r in range(n_reps):
            _body(nc, tc, *args, rep=r)
    if split_waits:
        _split_multi_waits(nc)
    return nc


def _body(nc, tc, ht, htkv, wq, wkv, wg, wo, eape, esink, gfix,
          cosq, sinq, cosk, sink, bandm, out, rep=0):
    cc_in = nc.dram_tensor(f"cc_in{rep}", [128, WL], F32)
    cc_out = nc.dram_tensor(f"cc_out{rep}", [GSZ, 128, WL], F32)

    with (
        tc.tile_pool(name="persist", bufs=1) as pp,
        tc.tile_pool(name="hslab", bufs=2) as hs,
    ):
        # ---- persistent SBUF state ----
        qT = [pp.tile([128, S], BF16, tag=f"qT{m}", name=f"qT{m}")
              for m in range(HPC)]
        kvlo = pp.tile([HD, HKV], BF16, tag="kvlo", name="kvlo")
        kvhi = pp.tile([HD, TL], BF16, tag="kvhi", name="kvhi")
        glo = pp.tile([HD, HKV], BF16, tag="glo", name="glo")
        ghi = pp.tile([HD, TL], BF16, tag="ghi", name="ghi")
        cosq_s = pp.tile([128, S], BF16, tag="cosq", name="cosq")
        sinq_s = pp.tile([128, S], BF16, tag="sinq", name="sinq")
        cosk_s = pp.tile([128, NW], BF16, tag="cosk", name="cosk")
        sink_s = pp.tile([128, NW], BF16, tag="sink", name="sink")
        eape_s = pp.tile([HD, 2 * RATIO], F32, tag="eape", name="eape")
        esink_s = pp.tile([1, HPC], F32, tag="esink", name="esink")
        gfix_s = pp.tile([HD, 1], F32, tag="gfix", name="gfix")
        bandm_s = pp.tile([WCH, TCH], BF16, tag="bandm", name="bandm")
        wo_s = pp.tile([HD, HPC, HID], BF16, tag="wo", name="wo")
        wq_s = pp.tile([128, KCH, CW], BF16, tag="wq", name="wq")
        wkv_s = pp.tile([128, KCH, 2 * HD], BF16, tag="wkv", name="wkv")
        wg_s = pp.tile([128, KCH, 2 * HD], BF16, tag="wg", name="wg")
        ones_w = pp.tile([WCH, 1], BF16, tag="ones_w", name="ones_w")
        ones_p = pp.tile([1, HD], F32, tag="ones_p", name="ones_p")
        kT = pp.tile([HD, NW], BF16, tag="kT", name="kT")
        v_s = pp.tile([WCH, NW // WCH, HD], BF16, tag="v", name="v")
        pg = pp.tile([128, NW], F32, tag="pg", name="pg")
        ident = pp.tile([128, 128], F32, tag="ident", name="ident")

        # DMA issue order is queue order: kv/gate weights + small tables
        # first so the PE can start within ~6us; bulky tables later.
        wkv_r = wkv.rearrange("(k p) c -> p k c", p=128)
        wg_r = wg.rearrange("(k p) c -> p k c", p=128)
        H = KCH // 2
        nc.sync.dma_start(wkv_s[:, 0:H], wkv_r[:, 0:H])
        nc.sync.dma_start(wkv_s[:, H:KCH], wkv_r[:, H:KCH])
        nc.sync.dma_start(eape_s[:], eape[:])
        nc.sync.dma_start(esink_s[:], esink[:])
        nc.sync.dma_start(gfix_s[:], gfix[:])
        nc.sync.dma_start(bandm_s[:], bandm[:])
        nc.vector.memset(ones_w[:], 1.0)
        nc.vector.memset(ones_p[:], 1.0)
        make_identity(nc, ident[:])

        def load_hsl(j):
            hsl = hs.tile([128, KCH, TCH], BF16, tag="hslab", name="hslab",
                          bufs=2)
            nc.sync.dma_start(
                hsl[:], ht[:, ts(j, TCH)].rearrange("(k p) t -> p k t", p=128))
            return hsl

        def qp_unit(psA, hsl, j, m, qp_bufs=1):
            """q projection for head m, chunk j, with fused interleaved RoPE."""
            tsl = ts(j, TCH)
            ps = psA.tile([128, TCH], F32, tag="psA", name="psA",
                          bufs=qp_bufs)
            for k in range(KCH):
                nc.tensor.matmul(ps[:], wq_s[:, k, ts(m, 128)], hsl[:, k, :],
                                 start=(k == 0), stop=(k == KCH - 1))
            nc.scalar.copy(qT[m][0:RD, tsl], ps[0:RD, :])
            rb = hs.tile([128, TCH], BF16, tag="ropebuf", name="ropebuf")
            nc.scalar.copy(rb[RD:128, :], ps[RD:128, :])
            sw = hs.tile([128, TCH], BF16, tag="ropeswap", name="ropeswap")
            nc.vector.stream_shuffle(sw[RD:128, :], rb[RD:128, :], _PAIR_SWAP)
            t1 = hs.tile([128, TCH], BF16, tag="ropet1", name="ropet1")
            nc.vector.tensor_mul(t1[RD:128, :], rb[RD:128, :],
                                 cosq_s[RD:128, tsl])
            t2 = hs.tile([128, TCH], BF16, tag="ropet2", name="ropet2")
            nc.vector.tensor_mul(t2[RD:128, :], sw[RD:128, :],
                                 sinq_s[RD:128, tsl])
            nc.vector.tensor_add(qT[m][RD:128, tsl], t1[RD:128, :],
                                 t2[RD:128, :])

        # =============== prologue scope ===============
        with (
            tc.tile_pool(name="psP", bufs=3, space="PSUM") as psP,
            tc.tile_pool(name="poolb", bufs=1) as pb,
        ):
            # ---- kv/gate shard projections (own slice + 4-token halo) ----
            hh = pb.tile([128, KCH, RATIO], BF16, tag="hhalo", name="hhalo")
            nc.sync.dma_start(
                hh[:], htkv[:, 0:RATIO].rearrange("(k p) t -> p k t", p=128))
            hks = []
            for e in range(TL // TCH):
                hk = hs.tile([128, KCH, TCH], BF16, tag="hslab", name="hslab",
                             bufs=2)
                hkr = htkv[:, RATIO + e * TCH: RATIO + (e + 1) * TCH
                           ].rearrange("(k p) t -> p k t", p=128)
                nc.sync.dma_start(hk[:, 0:H], hkr[:, 0:H])
                if e == 0:
                    nc.sync.dma_start(wg_s[:, 0:H], wg_r[:, 0:H])
                    nc.sync.dma_start(wg_s[:, H:KCH], wg_r[:, H:KCH])
                nc.sync.dma_start(hk[:, H:KCH], hkr[:, H:KCH])
                hks.append(hk)
            hsl_cache = {}
            for j in range(2):
                hsl_cache[j] = load_hsl(j)
            nc.sync.dma_start(wq_s[:], wq.rearrange("(k p) c -> p k c", p=128))
            for j in range(2, 4):
                hsl_cache[j] = load_hsl(j)

            def kvg_group(dst, col, wsrc, dslice, rhs):
                ps = psP.tile([128, TCH], F32, tag="psP", name="psP")
                n = rhs.shape[-1]
                for k in range(KCH):
                    nc.tensor.matmul(ps[:, 0:n], wsrc[:, k, col], rhs[:, k, :],
                                     start=(k == 0), stop=(k == KCH - 1))
                if dst is glo and dslice.start == 0:
                    nc.scalar.activation(dst[:, dslice], ps[:, 0:n],
                                         AF.Identity, bias=gfix_s[:, 0:1])
                else:
                    nc.scalar.copy(dst[:, dslice], ps[:, 0:n])

            # halo kv first (only needs wkv + the tiny halo slab)
            kvg_group(kvlo, slice(0, 128), wkv_s, slice(0, RATIO), hh)
            for e in range(TL // TCH):
                sl_lo = slice(RATIO + e * TCH, RATIO + (e + 1) * TCH)
                sl_hi = slice(e * TCH, (e + 1) * TCH)
                kvg_group(kvlo, slice(0, 128), wkv_s, sl_lo, hks[e])
                kvg_group(kvhi, slice(128, 256), wkv_s, sl_hi, hks[e])
                if e == 0:
                    kvg_group(glo, slice(0, 128), wg_s, slice(0, RATIO), hh)
                kvg_group(glo, slice(0, 128), wg_s, sl_lo, hks[e])
                kvg_group(ghi, slice(128, 256), wg_s, sl_hi, hks[e])

            # ---- overlap gated pooling over own 256 windows ----
            numer = pb.tile([HD, WL], F32, tag="numer", name="numer")
            denom = pb.tile([HD, WL], F32, tag="denom", name="denom")
            for half, (g_src, kv_src, acol, n) in enumerate(
                ((glo, kvlo, slice(0, RATIO), HKV),
                 (ghi, kvhi, slice(RATIO, 2 * RATIO), TL))
            ):
                e_t = pb.tile([HD, HKV], F32, tag="poole", name=f"poole{half}")
                nc.scalar.activation(e_t[:, 0:n], g_src[:, 0:n], AF.Exp)
                nc.vector.tensor_mul(
                    e_t[:, 0:n].rearrange("d (w r) -> d w r", r=RATIO),
                    e_t[:, 0:n].rearrange("d (w r) -> d w r", r=RATIO),
                    eape_s[:, None, acol].to_broadcast(
                        [HD, n // RATIO, RATIO]))
                ea = pb.tile([HD, HKV], F32, tag="poolea", name=f"poolea{half}")
                nc.vector.tensor_mul(ea[:, 0:n], e_t[:, 0:n], kv_src[:, 0:n])
                for acc, src in ((denom, e_t), (numer, ea)):
                    s3 = src[:, 0:TL].rearrange("d (w r) -> d w r", r=RATIO)
                    nm = f"poolred{half}{1 if acc is numer else 0}"
                    ra = pb.tile([HD, WL], F32, tag="poolra", name=nm + "a")
                    nc.vector.tensor_add(ra[:], s3[:, :, 0], s3[:, :, 1])
                    rc = pb.tile([HD, WL], F32, tag="poolrc", name=nm + "c")
                    nc.vector.tensor_add(rc[:], s3[:, :, 2], s3[:, :, 3])
                    if half == 0:
                        nc.vector.tensor_add(acc[:], ra[:], rc[:])
                    else:
                        nc.vector.tensor_add(ra[:], ra[:], rc[:])
                        nc.vector.tensor_add(acc[:], acc[:], ra[:])
            rec = pb.tile([HD, WL], F32, tag="poolrec", name="poolrec")
            nc.vector.reciprocal(rec[:], denom[:])
            pool_bf = pb.tile([HD, WL], F32, tag="pool_bf", name="pool_bf")
            nc.vector.tensor_mul(pool_bf[:], numer[:], rec[:])

            # ---- AllGather pooled windows (bf16) across the batch group ----
            nc.sync.dma_start(cc_in[:], pool_bf[:])
            nc.gpsimd.collective_compute(
                "AllGather", mybir.AluOpType.bypass,
                replica_groups=[[0, 1, 2, 3], [4, 5, 6, 7]],
                ins=[cc_in[:]], outs=[cc_out[:]],
            )

            # ---- q projections for chunks 0..3 (PE cover for the AG) ----
            nc.sync.dma_start(cosq_s[RD:128, :], cosq[:])
            nc.sync.dma_start(sinq_s[RD:128, :], sinq[:])
            nc.sync.dma_start(cosk_s[RD:128, :], cosk[:])
            nc.sync.dma_start(sink_s[RD:128, :], sink[:])
            nc.sync.dma_start(wo_s[:],
                              wo.rearrange("(h p) e -> p h e", p=HD))
            for j in range(4):
                for m in range(HPC):
                    qp_unit(psP, hsl_cache[j], j, m, qp_bufs=2)
            for j5 in (4, 5):
                hsl_cache[j5] = load_hsl(j5)
                for m in range(HPC):
                    qp_unit(psP, hsl_cache[j5], j5, m, qp_bufs=2)

            # ---- unpack AG, rope -> kT, transpose -> V ----
            for r in range(GSZ):
                nc.sync.dma_start(pg[:, ts(r, WL)], cc_out[r])
            nc.scalar.copy(kT[0:RD, :], pg[0:RD, :])
            krb = pb.tile([128, NW], BF16, tag="krope", name="krope")
            nc.scalar.copy(krb[RD:128, :], pg[RD:128, :])
            ksw = pb.tile([128, NW], BF16, tag="kswap", name="kswap")
            nc.vector.stream_shuffle(ksw[RD:128, :], krb[RD:128, :], _PAIR_SWAP)
            kt1 = pb.tile([128, NW], BF16, tag="kt1", name="kt1")
            nc.vector.tensor_mul(kt1[RD:128, :], krb[RD:128, :],
                                 cosk_s[RD:128, :])
            kt2 = pb.tile([128, NW], BF16, tag="kt2", name="kt2")
            nc.vector.tensor_mul(kt2[RD:128, :], ksw[RD:128, :],
                                 sink_s[RD:128, :])
            nc.vector.tensor_add(kT[RD:128, :], kt1[RD:128, :], kt2[RD:128, :])
            for wb in range(NW // WCH):
                tp = psP.tile([128, 128], F32, tag="vtrans", name="vtrans",
                              bufs=2)
                nc.tensor.transpose(tp[:], pg[:, ts(wb, 128)], ident[:])
                nc.scalar.copy(v_s[:, wb, :], tp[:])

        # =============== main loop ===============
        with (
            tc.tile_pool(name="psA", bufs=1, space="PSUM") as psA,
            tc.tile_pool(name="psS", bufs=2, space="PSUM") as psS,
            tc.tile_pool(name="psAcc", bufs=2, space="PSUM") as pacc,
            tc.tile_pool(name="psDen", bufs=1, space="PSUM") as pden,
            tc.tile_pool(name="psO", bufs=2, space="PSUM") as pso,
            tc.tile_pool(name="pt", bufs=4) as ptp,
            tc.tile_pool(name="att", bufs=2) as att,
        ):
            hsl_cache[6] = load_hsl(6)

            attnT = {}         # (j, h) -> a_sb tile
            pending = None     # deferred epilogue context

            def at_stream(j, h, fillers=()):
                fillers = list(fillers)
                qsl = qT[h][:, ts(j, TCH)]
                den_ps = pden.tile([1, TCH], F32, tag="den", name="den")
                acc_ps = pacc.tile([128, TCH], F32, tag="acc", name="acc")
                s_tiles = {}

                def score(wc):
                    sp = psS.tile([WCH, TCH], F32, tag="s", name="s")
                    nc.tensor.matmul(sp[:], kT[:, ts(wc, WCH)], qsl,
                                     start=True, stop=True)
                    s_tiles[wc] = sp

                # diagonal w-chunk first: its DVE mask-multiply runs while
                # the DVE queue is still empty, not behind filler evacuations
                wcs = [j] + list(range(j))
                score(wcs[0])
                half = (len(wcs) + 1) // 2
                pend = None
                den_i = 0
                for i, wc in enumerate(wcs):
                    if i + 1 < len(wcs):
                        score(wcs[i + 1])
                    sp = s_tiles.pop(wc)
                    pt = ptp.tile([WCH, TCH], BF16, tag="pt", name="pt")
                    nc.scalar.activation(pt[:], sp[:], AF.Exp, scale=SCALE)
                    if wc == j:
                        ptm = ptp.tile([WCH, TCH], BF16, tag="ptm", name="ptm")
                        nc.vector.tensor_mul(ptm[:], pt[:], bandm_s[:])
                        pt = ptm
                    # pair exp tiles on DVE so the ones-matmul runs per pair
                    if pend is None and i < len(wcs) - 1:
                        pend = pt
                    else:
                        if pend is not None:
                            pr = ptp.tile([WCH, TCH], BF16, tag="ptpair",
                                          name="ptpair")
                            nc.vector.tensor_add(pr[:], pend[:], pt[:])
                            pend = None
                        else:
                            pr = pt
                        nc.tensor.matmul(den_ps[:], ones_w[:], pr[:],
                                         start=(den_i == 0),
                                         stop=(den_i == half - 1))
                        den_i += 1
                    nc.tensor.matmul(acc_ps[:], v_s[:, wc, :], pt[:],
                                     start=(i == 0), stop=(i == len(wcs) - 1))
                    if i >= 1 and fillers:
                        fillers.pop(0)()
                for f in fillers:
                    f()
                # early (ACT/DVE) part of the epilogue; PE part is deferred
                den_sb = att.tile([1, TCH], F32, tag="den_sb", name="den_sb")
                nc.scalar.activation(den_sb[:], den_ps[:], AF.Identity,
                                     bias=esink_s[0:1, h:h + 1])
                rec_sb = att.tile([1, TCH], F32, tag="rec_sb", name="rec_sb")
                nc.vector.reciprocal(rec_sb[:], den_sb[:])
                return (j, h, acc_ps, rec_sb)

            def at_epilogue(ctx):
                j, h, acc_ps, rec_sb = ctx
                rb_ps = pso.tile([128, TCH], F32, tag="o", name="o")
                nc.tensor.matmul(rb_ps[:], ones_p[:], rec_sb[:],
                                 start=True, stop=True)
                rb_sb = att.tile([128, TCH], F32, tag="rb_sb", name="rb_sb")
                nc.vector.tensor_copy(rb_sb[:], rb_ps[:])
                a_sb = att.tile([128, TCH], BF16, tag=f"attnT{h}",
                                name=f"attnT{h}")
                nc.vector.tensor_mul(a_sb[:], acc_ps[:], rb_sb[:])
                attnT[(j, h)] = a_sb

            def op_group(j, idx):
                tt, e = divmod(idx, HID // TCH)
                o_ps = pso.tile([128, TCH], F32, tag="o", name="o")
                for h in range(HPC):
                    nc.tensor.matmul(o_ps[:],
                                     attnT[(j, h)][:, ts(tt, 128)],
                                     wo_s[:, h, ts(e, TCH)],
                                     start=(h == 0), stop=(h == HPC - 1))
                o_sb = att.tile([128, TCH], BF16, tag="o_sb", name="o_sb")
                if e % 2 == 0:
                    nc.scalar.copy(o_sb[:], o_ps[:])
                else:
                    nc.vector.tensor_copy(o_sb[:], o_ps[:])
                nc.sync.dma_start(
                    out[j * TCH + tt * 128:j * TCH + (tt + 1) * 128,
                        ts(e, TCH)], o_sb[:])

            def op_quarter(j, qh):
                for idx in range(4 * qh, 4 * qh + 4):
                    op_group(j, idx)

            qp_next = 6 * HPC   # flat index of next q-proj unit (chunk*4+m)
            for j in range(NCH):
                if j + 7 < NCH:
                    hsl_cache[j + 7] = load_hsl(j + 7)
                for h in range(HPC):
                    fillers = []
                    if pending is not None:
                        fillers.append(
                            lambda ctx=pending: at_epilogue(ctx))
                    if j >= 1:
                        fillers += [
                            (lambda jj=j - 1, ii=i: op_group(jj, ii))
                            for i in range(4 * h, 4 * h + 4)]
                    pending = at_stream(j, h, fillers)
                    if qp_next < NCH * HPC:
                        j2, m = divmod(qp_next, HPC)
                        qp_unit(psA, hsl_cache[j2], j2, m)
                        qp_next += 1
                for j2 in list(hsl_cache):
                    if (j2 + 1) * HPC <= qp_next:
                        hsl_cache.pop(j2)
            at_epilogue(pending)
            op_quarter(NCH - 1, 0)
            op_quarter(NCH - 1, 1)
            op_quarter(NCH - 1, 2)
            op_quarter(NCH - 1, 3)


_WS_CTR = [0]


def _split_multi_waits(nc):
    """This walrus build accepts at most ONE sync wait per instruction; hoist
    extras onto same-engine NOPs placed immediately before."""
    f = nc.m.functions[0]
    for blk in f.blocks:
        insts = blk.instructions
        if not any(i.sync_info is not None and len(i.sync_info.on_wait) > 1
                   for i in insts):
            continue
        new_list = []
        for inst in insts:
            si = inst.sync_info
            if si is not None and len(si.on_wait) > 1:
                waits = list(si.on_wait)
                for w in waits[:-1]:
                    _WS_CTR[0] += 1
                    new_list.append(mybir.InstNoOp(
                        name=f"waitsplit-{_WS_CTR[0]}",
                        engine=inst.engine,
                        bass_nofuse=True,
                        sync_info=mybir.SyncInfo(on_wait=[w], on_update=[])))
                inst.sync_info = mybir.SyncInfo(
                    on_wait=[waits[-1]], on_update=list(si.on_update))
            new_list.append(inst)
        blk.instructions = new_list


# ---------------------------------------------------------------------------
# host side
# ---------------------------------------------------------------------------

def _rope_tables(positions):
    half = RD // 2
    inv_freq = 1.0 / (THETA ** (np.arange(half, dtype=np.float64) / half))
    ang = positions[None, :].astype(np.float64) * inv_freq[:, None]  # [32, L]
    cos_t = np.repeat(np.cos(ang), 2, axis=0).astype(np.float32)
    sin_t = np.repeat(np.sin(ang), 2, axis=0).astype(np.float32)
    sin_t[0::2] *= -1.0                                  # a-rows get -sin
    return cos_t, sin_t


def _prep_inputs(hidden, wq, wkv, wgate, ape, sinks, wo):
    bf = ml_dtypes.bfloat16
    cosq_t, sinq_t = _rope_tables(np.arange(S))
    cosk_t, sink_t = _rope_tables(np.arange(NW) * RATIO)
    pw, ft = np.meshgrid(np.arange(WCH), np.arange(TCH), indexing="ij")
    band = (ft >= RATIO * pw + RATIO - 1).astype(np.float32)     # [WCH, TCH]
    eape = np.empty((HD, 2 * RATIO), np.float32)
    for r in range(RATIO):
        eape[:, r] = np.exp(ape[r, :HD])
        eape[:, RATIO + r] = np.exp(ape[r, HD:])
    maps = []
    for c in range(N_CORES):
        b, g = divmod(c, GSZ)
        htc = np.ascontiguousarray(hidden[b].T).astype(bf)     # [HID, S]
        htkv = np.zeros((HID, HKV), bf)
        lo = TL * g - RATIO
        htkv[:, (RATIO if g == 0 else 0):] = htc[:, max(lo, 0):TL * (g + 1)]
        maps.append({
            "ht": htc,
            "htkv": htkv,
            "wq": np.ascontiguousarray(wq[:, g * CW:(g + 1) * CW]).astype(bf),
            "wkv": wkv.astype(bf),
            "wg": wgate.astype(bf),
            "wo": np.ascontiguousarray(wo[g * CW:(g + 1) * CW, :]).astype(bf),
            "eape": eape,
            "esink": np.exp(sinks[g * HPC:(g + 1) * HPC]).astype(
                np.float32).reshape(1, HPC),
            "gfix": np.full((HD, 1), -30000.0 if g == 0 else 0.0, np.float32),
            "cosq": cosq_t.astype(bf), "sinq": sinq_t.astype(bf),
            "cosk": cosk_t.astype(bf), "sink": sink_t.astype(bf),
            "bandm": band.astype(bf),
        })
    return maps


_RUNNER_CACHE = {}


def _get_runner(n_reps: int = 1):
    if n_reps in _RUNNER_CACHE:
        return _RUNNER_CACHE[n_reps]
    import jax
    from jax.sharding import Mesh, PartitionSpec
    from jax.experimental.shard_map import shard_map
    from concourse.bass2jax import (_bass_exec_p, install_neuronx_cc_hook,
                                    partition_id_tensor)

    nc = _build_nc(n_reps)
    install_neuronx_cc_hook()
    partition_name = nc.partition_id_tensor.name if nc.partition_id_tensor else None
    in_names, out_names, out_avals, zero_outs = [], [], [], []
    for alloc in nc.m.functions[0].allocations:
        if not isinstance(alloc, mybir.MemoryLocationSet):
            continue
        name = alloc.memorylocations[0].name
        if alloc.kind == "ExternalInput":
            if name != partition_name:
                in_names.append(name)
        elif alloc.kind == "ExternalOutput":
            out_names.append(name)
            shape = tuple(alloc.tensor_shape)
            dtype = mybir.dt.np(alloc.dtype)
            out_avals.append(jax.core.ShapedArray(shape, dtype))
            zero_outs.append(np.zeros(shape, dtype))
    n_params = len(in_names)
    all_in_names = list(in_names) + out_names
    if partition_name is not None:
        all_in_names.append(partition_name)

    def _kernel_body(*args):
        operands = list(args)
        if partition_name is not None:
            operands.append(partition_id_tensor())
        outs = _bass_exec_p.bind(
            *operands,
            out_avals=tuple(out_avals),
            in_names=tuple(all_in_names),
            out_names=tuple(out_names),
            lowering_input_output_aliases=(),
            sim_require_finite=True,
            sim_require_nnan=True,
            nc=nc,
        )
        return tuple(outs)

    devices = jax.devices()[:N_CORES]
    mesh = Mesh(np.asarray(devices), ("core",))
    spec = PartitionSpec("core")
    fn = jax.jit(shard_map(
        _kernel_body, mesh=mesh,
        in_specs=(spec,) * (n_params + len(out_names)),
        out_specs=(spec,) * len(out_names), check_rep=False))
    runner = (fn, in_names, out_names, zero_outs, mesh)
    _RUNNER_CACHE[n_reps] = runner
    return runner


def _run_core_maps(maps, n_reps: int = 1):
    import jax
    from jax.sharding import NamedSharding, PartitionSpec
    fn, in_names, out_names, zero_outs, mesh = _get_runner(n_reps)
    sh = NamedSharding(mesh, PartitionSpec("core"))
    args = [jax.device_put(
        np.concatenate([np.asarray(m[name]) for m in maps], axis=0), sh)
        for name in in_names]
    for z in zero_outs:
        args.append(jax.device_put(
            np.zeros((N_CORES * z.shape[0], *z.shape[1:]), z.dtype), sh))
    res = fn(*args)
    jax.block_until_ready(res)
    return np.asarray(res[0]).astype(np.float32).reshape(N_CORES, S, HID)


def kernel(hidden, wq, wkv, wgate, ape, sinks, wo,
           ratio=RATIO, head_dim=HD, rope_head_dim=RD, num_heads=NH):
    hidden = np.asarray(hidden, np.float32)
    maps = _prep_inputs(hidden, np.asarray(wq, np.float32),
                        np.asarray(wkv, np.float32),
                        np.asarray(wgate, np.float32),
                        np.asarray(ape, np.float32),
                        np.asarray(sinks, np.float32),
                        np.asarray(wo, np.float32))
    partials = _run_core_maps(maps)
    out = np.empty((B, S, HID), np.float32)
    for b in range(B):
        out[b] = partials[b * HPC:(b + 1) * HPC].astype(np.float64).sum(
            axis=0).astype(np.float32)
    return out
